# revision 39
# baseline (speedup 1.0000x reference)
"""Trainium2 Bass kernel for nn_DifferentiableCGCNN (N=4096 atoms, 8 NeuronCores).

v2 strategy (SPMD, one identical program per core, no collectives):
  - neighbor SELECTION by surrogate rank only (validated offline: end-to-end
    output shift 5e-4 << 2e-2 tol): torus surrogate dots as bf16 PE matmuls,
    top-8 per 512-block (max/max_index), then top-12 of the 64 candidates
    by value; global indices via compare-select masks.
  - exact distances only for the selected 12 (1.5MB gather of padded fracs
    rows), d12 = sqrt(e^T G e) on DVE/ACT.
  - embedding: host-transposed species logits -> Exp -> row matmuls with
    [embw|ones] so the softmax sum rides along as column 64; normalization
    folded into the PSUM evacuation (x * 1/sum + bias). bf16 atom-feature
    table [N,128] written to DRAM for the neighbor gather. Local shard is
    handled feature-major from a per-core sploclT input; the sum row rides
    as partition 64 so the bias folds exactly ((af_un@W + rs*b) * 1/rs).
  - neighbor features: transpose-mode dma_gather (256B bf16 rows) lands
    af_j^T directly as matmul lhsT in partitions 0:64; gaussian features are
    built into partitions 64:128 of the same tile; conv z = one
    [128]-contraction bf16 matmul per (b,m) + hi via identity matmul.
  - LN via bn_stats; sigmoid on ACT; softplus = relu(x) + ln(1+exp(-|x|))
    using Abs/Exp/Ln; ops batched per layer to minimize act-table loads;
    elementwise in bf16 where the DVE 2x mode applies.
  - final occupancy pooling + fc on host from per-core [512,64] outputs.
"""
import os
import sys

import numpy as np

for _p in ("/opt/trn_rl_repo",):
    if os.path.isdir(_p) and _p not in sys.path:
        sys.path.insert(0, _p)

N = 4096
NCORES = 8
NL = N // NCORES          # 512 atoms per core
NB = NL // 128            # 4 row tiles per core
SPECIES = 100
FEA = 64
KG = 64                   # gaussian filters
M = 12                    # neighbors
BLK = 512                 # surrogate block width
NBLK = N // BLK           # 8 blocks
CAND = NBLK * 8           # 64 candidates per row
LN_EPS = 1e-5
BIG = 1e30

OFFSET = np.linspace(0.0, 8.0, KG).astype(np.float32)
COEFF = float(-0.5 / (8.0 / (KG - 1)) ** 2)

_cache = {}


def _build_program(debug: bool = False):
    from contextlib import ExitStack

    import concourse.bacc as bacc
    import concourse.mybir as mybir
    from concourse.tile import TileContext

    dt = mybir.dt
    AF = mybir.ActivationFunctionType
    ALU = mybir.AluOpType
    AX = mybir.AxisListType
    f32 = dt.float32
    bf16 = dt.bfloat16
    f32r = dt.float32r

    nc = bacc.Bacc("TRN2", target_bir_lowering=False, debug=False,
                   enable_asserts=False)

    # ---- dram inputs ----
    d_splogT = nc.dram_tensor("splogT", [128, N], f32, kind="ExternalInput")
    d_sploclT = nc.dram_tensor("sploclT", [128, NL], f32,
                               kind="ExternalInput")
    d_fracsT = nc.dram_tensor("fracsT", [3, N], f32, kind="ExternalInput")
    d_fl = nc.dram_tensor("fl", [NL, 3], f32, kind="ExternalInput")
    d_flT = nc.dram_tensor("flT", [3, NL], f32, kind="ExternalInput")
    d_frecs = nc.dram_tensor("frecs", [N, 64], f32, kind="ExternalInput")
    d_embwx = nc.dram_tensor("embwx", [SPECIES, FEA + 1], f32,
                             kind="ExternalInput")
    d_embbrow = nc.dram_tensor("embbrow", [128, FEA], f32,
                               kind="ExternalInput")
    d_wib1 = nc.dram_tensor("wib1", [FEA + 1, 2 * FEA], f32,
                            kind="ExternalInput")
    d_wib2 = nc.dram_tensor("wib2", [FEA + 1, 2 * FEA], f32,
                            kind="ExternalInput")
    d_wjx = nc.dram_tensor("wjx", [FEA, 4 * FEA], bf16, kind="ExternalInput")
    d_wnx = nc.dram_tensor("wnx", [FEA, 4 * FEA], bf16, kind="ExternalInput")
    d_gcol = nc.dram_tensor("gcol", [128, 6], f32, kind="ExternalInput")
    d_wroot = nc.dram_tensor("wroot", [3, 1], f32, kind="ExternalInput")
    d_noff = nc.dram_tensor("noff", [KG, 1], f32, kind="ExternalInput")
    d_selfid = nc.dram_tensor("selfid", [128, NB], f32, kind="ExternalInput")
    d_blockoff = nc.dram_tensor("blockoff", [128, CAND], dt.uint32,
                                kind="ExternalInput")
    d_identb = nc.dram_tensor("identb", [128, 128], bf16, kind="ExternalInput")
    d_ident = nc.dram_tensor("ident", [128, 128], f32, kind="ExternalInput")

    d_out = nc.dram_tensor("atom2", [NL, FEA], f32, kind="ExternalOutput")

    dbg = {}
    if debug:
        def dbg_t(name, shape, dtyp=f32):
            dbg[name] = nc.dram_tensor("dbg_" + name, shape, dtyp,
                                       kind="ExternalOutput")
        dbg_t("mxall", [128, NB * CAND])
        dbg_t("nidx", [128, NB * M], dt.uint32)
        dbg_t("d12", [128, NB * M])
        dbg_t("af0", [128, FEA])
        dbg_t("hi1", [128, 2 * FEA])
        dbg_t("hjg", [128, 512])
        dbg_t("gauss", [KG, 512])
        dbg_t("z1", [128, M * 128])
        dbg_t("atom1", [NL, FEA])

    def _body():  # noqa: PLR0915
        with TileContext(nc) as tc, \
             nc.allow_low_precision(reason="bf16 staging is intentional"):
            with ExitStack() as ctx:
                const = ctx.enter_context(tc.tile_pool(name="const", bufs=1))
                persist = ctx.enter_context(tc.tile_pool(name="persist",
                                                         bufs=1))
                work = ctx.enter_context(tc.tile_pool(name="work", bufs=3))
                spool = ctx.enter_context(tc.tile_pool(name="spool", bufs=1))
                epool = ctx.enter_context(tc.tile_pool(name="epool", bufs=1))
                ework = ctx.enter_context(tc.tile_pool(name="ework", bufs=3))
                gpool = ctx.enter_context(tc.tile_pool(name="gpool", bufs=1))
                cvp = ctx.enter_context(tc.tile_pool(name="cvp", bufs=1))
                dbp = ctx.enter_context(tc.tile_pool(name="dbp", bufs=1))
                drp = ctx.enter_context(tc.tile_pool(name="dram", bufs=1,
                                                     space="DRAM"))

                def cload(dram, shape, dtyp=f32):
                    t = const.tile(shape, dtyp, tag=dram.name + "_c")
                    nc.sync.dma_start(t[:], dram.ap())
                    return t

                embwx = cload(d_embwx, [SPECIES, FEA + 1])
                embbrow = cload(d_embbrow, [128, FEA])
                wib1 = cload(d_wib1, [FEA + 1, 2 * FEA])
                wib2 = cload(d_wib2, [FEA + 1, 2 * FEA])
                wjx = cload(d_wjx, [FEA, 4 * FEA], bf16)
                wnx = cload(d_wnx, [FEA, 4 * FEA], bf16)
                gcol = cload(d_gcol, [128, 6])
                wroot = cload(d_wroot, [3, 1])
                gsq = const.tile([3, 1], f32, tag="gsq")
                nc.vector.tensor_tensor(gsq[:], wroot[:], wroot[:], ALU.mult)
                noff = cload(d_noff, [KG, 1])
                selfid = cload(d_selfid, [128, NB])
                blockoff = cload(d_blockoff, [128, CAND], dt.uint32)
                identb = cload(d_identb, [128, 128], bf16)
                ident = cload(d_ident, [128, 128])
                mpi = const.tile([3, 1], f32, tag="mpi")
                nc.vector.memset(mpi[:], -np.pi)
                hpi = const.tile([3, 1], f32, tag="hpi")
                nc.vector.memset(hpi[:], np.pi / 2)
                epsc = const.tile([128, 1], f32, tag="epsc")
                nc.vector.memset(epsc[:], LN_EPS)

                # dram scratch
                hjw = drp.tile([N, 4 * FEA], bf16, tag="hjw")
                dflat2 = drp.tile([NB * M, 128], f32, tag="dflat2")
                hbmI = drp.tile([16, NB * M * 8], dt.int16, tag="hbmI")

                # persistent cross-stage tiles
                nidx16 = persist.tile([128, NB * M], dt.int16, tag="nidx16")
                d12 = persist.tile([128, NB * M], f32, tag="d12")
                hjwg = persist.tile([128, NB * M * 4 * FEA], bf16,
                                    tag="hjwg")
                gss = persist.tile([KG, NB * M * 128], bf16, tag="gss")
                flb = [persist.tile([128, 3], f32, tag=f"flb_{b}",
                                    name=f"flb_{b}") for b in range(NB)]
                rrloc = [persist.tile([128, 1], f32, tag=f"rr_{b}",
                                      name=f"rr_{b}") for b in range(NB)]
                atom0 = [persist.tile([128, FEA], f32, tag=f"a0_{b}",
                                      name=f"a0_{b}") for b in range(NB)]
                atom1 = [persist.tile([128, FEA], f32, tag=f"a1_{b}",
                                      name=f"a1_{b}") for b in range(NB)]
                atom2 = [persist.tile([128, FEA], f32, tag=f"a2_{b}",
                                      name=f"a2_{b}") for b in range(NB)]
                hi1 = [persist.tile([128, 2 * FEA], bf16, tag=f"hi1_{b}",
                                    name=f"hi1_{b}") for b in range(NB)]
                hi2 = [persist.tile([128, 2 * FEA], bf16, tag=f"hi2_{b}",
                                    name=f"hi2_{b}") for b in range(NB)]

                # ======== stages S (select) + E (embed) + G (gather) ======
                uT = spool.tile([36, N], bf16, tag="uT")
                uloc = spool.tile([36, NL], bf16, tag="uloc")
                fT = spool.tile([3, N], f32, tag="fT")
                fTl = spool.tile([3, NL], f32, tag="fTl")
                mxall = spool.tile([128, NB * CAND], f32, tag="mxall")
                idxgF = spool.tile([128, NB * CAND], f32, tag="idxgF")
                idxg = spool.tile([128, NB * CAND], dt.uint32, tag="idxg")
                vals = spool.tile([128, NB * 16], f32, tag="vals")
                nidxF = spool.tile([128, NB * M], f32, tag="nidxF")
                msk = spool.tile([128, NB * M * CAND], f32, tag="msk")

                expT = epool.tile([128, N], f32, tag="expT")
                explT = epool.tile([128, NL], f32, tag="explT")
                afTx = epool.tile([FEA + 1, NL], f32, tag="afTx")

                with tc.tile_pool(name="psS", bufs=2, space="PSUM") as psS, \
                     tc.tile_pool(name="psA", bufs=1, space="PSUM") as psA, \
                     tc.tile_pool(name="psL", bufs=1, space="PSUM") as psL, \
                     tc.tile_pool(name="psT", bufs=1, space="PSUM") as psT:

                    # ---- stage S ----
                    nc.sync.dma_start(fT[:], d_fracsT.ap())
                    nc.sync.dma_start(fTl[:], d_flT.ap())
                    for b in range(NB):
                        nc.sync.dma_start(flb[b][:],
                                          d_fl.ap()[b * 128:(b + 1) * 128, :])

                    # u rows: cos at partitions 0-2, sin at 32-34. Only
                    # the local (stationary) side carries the G_aa scaling
                    # so the [*, N]-wide scaling ops disappear.
                    nc.vector.memset(uT[:], 0.0)
                    nc.scalar.activation(uT[32:35, :], fT[:], AF.Sin,
                                         scale=2 * np.pi, bias=mpi[:])
                    nc.scalar.activation(fT[:], fT[:], AF.Abs,
                                         scale=2 * np.pi, bias=mpi[:])
                    nc.scalar.activation(uT[0:3, :], fT[:], AF.Sin,
                                         scale=-1.0, bias=hpi[:])
                    ulf = spool.tile([36, NL], f32, tag="ulf")
                    nc.vector.memset(ulf[:], 0.0)
                    nc.scalar.activation(ulf[32:35, :], fTl[:], AF.Sin,
                                         scale=2 * np.pi, bias=mpi[:])
                    nc.scalar.activation(fTl[:], fTl[:], AF.Abs,
                                         scale=2 * np.pi, bias=mpi[:])
                    nc.scalar.activation(ulf[0:3, :], fTl[:], AF.Sin,
                                         scale=-1.0, bias=hpi[:])
                    nc.vector.tensor_scalar_mul(ulf[0:3, :], ulf[0:3, :],
                                                gsq[:])
                    nc.vector.tensor_scalar_mul(ulf[32:35, :],
                                                ulf[32:35, :], gsq[:])
                    nc.vector.tensor_copy(uloc[:], ulf[:])

                    idxg_v = idxg[:].rearrange("p (b c) -> p b c", b=NB)
                    mx_v = mxall[:].rearrange("p (b c) -> p b c", b=NB)

                    for b in range(NB):
                        for j in range(8):
                            dps = psS.tile([128, 512], f32, tag="dots")
                            nc.tensor.matmul(
                                dps[:], uloc[:, b * 128:(b + 1) * 128],
                                uT[:, j * 512:(j + 1) * 512],
                                start=True, stop=True)
                            nc.vector.max(
                                out=mx_v[:, b, j * 8:(j + 1) * 8], in_=dps[:])
                            nc.vector.max_index(
                                idxg_v[:, b, j * 8:(j + 1) * 8],
                                mx_v[:, b, j * 8:(j + 1) * 8], dps[:])

                    nc.vector.tensor_tensor(
                        idxg_v, idxg_v,
                        blockoff[:].unsqueeze(1).to_broadcast([128, NB, CAND]),
                        ALU.add)
                    nc.vector.tensor_copy(idxgF[:], idxg[:])

                    # self-exclusion on candidate values
                    smsk = work.tile([128, NB * CAND], f32, tag="smsk")
                    nc.vector.tensor_tensor(
                        smsk[:].rearrange("p (b c) -> p b c", b=NB),
                        idxgF[:].rearrange("p (b c) -> p b c", b=NB),
                        selfid[:].unsqueeze(2).to_broadcast([128, NB, CAND]),
                        ALU.is_equal)
                    nc.vector.scalar_tensor_tensor(mxall[:], smsk[:], -BIG,
                                                   mxall[:], ALU.mult,
                                                   ALU.add)
                    if debug:
                        nc.sync.dma_start(dbg["mxall"].ap(), mxall[:])

                    # top-12 of 64 per tile by value
                    for b in range(NB):
                        seg = mxall[:, b * CAND:(b + 1) * CAND]
                        nc.vector.max(out=vals[:, b * 16:b * 16 + 8], in_=seg)
                        mr = work.tile([128, CAND], f32, tag="mr")
                        nc.vector.match_replace(
                            out=mr[:],
                            in_to_replace=vals[:, b * 16:b * 16 + 8],
                            in_values=seg, imm_value=-BIG)
                        nc.vector.max(out=vals[:, b * 16 + 8:b * 16 + 16],
                                      in_=mr[:])

                    # global indices of the selected 12 via compare-select
                    vals_v = vals[:].rearrange("p (b v) -> p b v", b=NB)
                    msk_v = msk[:].rearrange("p (b m c) -> p b m c", b=NB,
                                             m=M)
                    nc.vector.tensor_tensor(
                        msk_v,
                        mxall[:].rearrange("p (b c) -> p b c", b=NB)
                        .unsqueeze(2).to_broadcast([128, NB, M, CAND]),
                        vals_v[:, :, 0:M].unsqueeze(3)
                        .to_broadcast([128, NB, M, CAND]),
                        ALU.is_equal)
                    nc.vector.tensor_tensor(
                        msk_v, msk_v,
                        idxgF[:].rearrange("p (b c) -> p b c", b=NB)
                        .unsqueeze(2).to_broadcast([128, NB, M, CAND]),
                        ALU.mult)
                    nc.vector.tensor_reduce(
                        nidxF[:].rearrange("p (b m) -> p b m", b=NB), msk_v,
                        axis=AX.X, op=ALU.max)
                    nc.vector.tensor_copy(nidx16[:], nidxF[:])
                    if debug:
                        nidxU = dbp.tile([128, NB * M], dt.uint32,
                                          tag="nidxU")
                        nc.vector.tensor_copy(nidxU[:], nidxF[:])
                        nc.sync.dma_start(dbg["nidx"].ap(), nidxU[:])

                    # ---- stage E (independent of S; overlaps it) ----
                    for h in range(8):
                        sl = slice(h * 512, (h + 1) * 512)
                        nc.sync.dma_start(expT[:, sl], d_splogT.ap()[:, sl])
                        nc.scalar.activation(expT[0:SPECIES, sl],
                                             expT[0:SPECIES, sl], AF.Exp)
                    nc.sync.dma_start(explT[:], d_sploclT.ap())
                    nc.scalar.activation(explT[0:SPECIES, :],
                                         explT[0:SPECIES, :], AF.Exp)

                    # full-graph neighbor table: hjW rows =
                    # af_row @ [wj1|wj2]  (af normalized + emb-biased)
                    for c in range(N // 128):
                        ps = psA.tile([128, FEA + 1], f32, tag="psA")
                        nc.tensor.matmul(
                            ps[:], expT[0:SPECIES, c * 128:(c + 1) * 128],
                            embwx[:], start=True, stop=True)
                        rr = ework.tile([128, 1], f32, tag="rr")
                        nc.vector.reciprocal(rr[:], ps[:, FEA:FEA + 1])
                        ab = ework.tile([128, FEA], bf16, tag="ab")
                        nc.vector.scalar_tensor_tensor(
                            ab[:], ps[:, 0:FEA], rr[:], embbrow[:],
                            ALU.mult, ALU.add)
                        tp2 = psA.tile([FEA, 128], bf16, tag="tp2")
                        nc.tensor.transpose(tp2[:], ab[:], identb[:])
                        abT = ework.tile([FEA, 128], bf16, tag="abT")
                        nc.vector.tensor_copy(abT[:], tp2[:])
                        psW = psA.tile([128, 4 * FEA], f32, tag="psW")
                        nc.tensor.matmul(psW[:], abT[:], wjx[:],
                                         start=True, stop=True)
                        hw_ = ework.tile([128, 4 * FEA], bf16, tag="hw_")
                        nc.vector.tensor_copy(hw_[:], psW[:])
                        nc.sync.dma_start(hjw[c * 128:(c + 1) * 128, :],
                                          hw_[:])

                    # local feature-major [af_unnorm ; sums] via f32r matmuls
                    psl = psL.tile([FEA + 1, NL], f32, tag="psl")
                    nc.tensor.matmul(psl[0:FEA, :],
                                     embwx[:, 0:FEA],
                                     explT[0:SPECIES, :],
                                     start=True, stop=True)
                    nc.tensor.matmul(psl[FEA:FEA + 1, :],
                                     embwx[:, FEA:FEA + 1],
                                     explT[0:SPECIES, :],
                                     start=True, stop=True)
                    nc.scalar.activation(afTx[:], psl[:], AF.Copy)

                    # per-tile: rows (atom0, rr) via PE transpose; hi1 matmul
                    for b in range(NB):
                        sl = slice(b * 128, (b + 1) * 128)
                        tp = psT.tile([128, FEA + 1], f32, tag="tpE")
                        nc.tensor.transpose(tp[:], afTx[:, sl],
                                            ident[0:FEA + 1, 0:FEA + 1])
                        nc.vector.reciprocal(rrloc[b][:], tp[:, FEA:FEA + 1])
                        nc.vector.scalar_tensor_tensor(
                            atom0[b][:], tp[:, 0:FEA], rrloc[b][:],
                            embbrow[:], ALU.mult, ALU.add)
                        hp = psT.tile([128, 2 * FEA], f32, tag="hpE")
                        nc.tensor.matmul(hp[:], afTx[:, sl], wib1[:],
                                         start=True, stop=True)
                        nc.vector.tensor_scalar_mul(hi1[b][:], hp[:],
                                                    rrloc[b][:])
                    if debug:
                        nc.sync.dma_start(dbg["af0"].ap(), atom0[0][:])
                        hj1f = dbp.tile([128, 2 * FEA], f32, tag="hj1f")
                        nc.vector.tensor_copy(hj1f[:], hi1[0][:])
                        nc.sync.dma_start(dbg["hi1"].ap(), hj1f[:])

                    # ---- stage G: gathers + d12 + gauss ----
                    hv = hbmI[:].rearrange("s (c e) -> s c e", e=8)
                    for w in range(8):
                        nc.sync.dma_start(hv[:, :, w],
                                          nidx16[16 * w:16 * (w + 1), :])
                    idxsG = gpool.tile([128, NB * M * 8], dt.int16,
                                       tag="idxsG")
                    for r in range(8):
                        nc.sync.dma_start(idxsG[16 * r:16 * (r + 1), :],
                                          hbmI[:])

                    # fracs gather: [128, 48, 64] f32 (256B rows)
                    crec = gpool.tile([128, NB * M * 64], f32, tag="crec")
                    crec_ch = crec[:].rearrange("p (c e) -> p c e", e=64)
                    for k in range(6):
                        nc.gpsimd.dma_gather(
                            crec_ch[:, k * 8:(k + 1) * 8, :], d_frecs.ap(),
                            idxsG[:, k * 64:(k + 1) * 64], 1024, 1024, 64)

                    # neighbor z-contribution gather (512B bf16 rows,
                    # atom-major — added to PSUM via one identity matmul)
                    hjwg_ch = hjwg[:].rearrange("p (c e) -> p c e",
                                                e=4 * FEA)
                    for k in range(6):
                        nc.gpsimd.dma_gather(
                            hjwg_ch[:, k * 8:(k + 1) * 8, :], hjw[:],
                            idxsG[:, k * 64:(k + 1) * 64],
                            1024, 1024, 4 * FEA)
                    if debug:
                        hjdbg = dbp.tile([128, 512], f32, tag="hjdbg")
                        nc.vector.tensor_copy(hjdbg[:], hjwg[:, 0:512])
                        nc.sync.dma_start(dbg["hjg"].ap(), hjdbg[:])

                    # ---- d12 (exact metric for the selected 12) ----
                    da = [gpool.tile([128, NB * M], f32, tag=f"da{a}",
                                     name=f"da{a}") for a in range(3)]
                    for b in range(NB):
                        for a in range(3):
                            nc.vector.tensor_scalar(
                                da[a][:, b * M:(b + 1) * M],
                                crec_ch[:, b * M:(b + 1) * M, a],
                                flb[b][:, a:a + 1], None, op0=ALU.subtract)
                    W = NB * M
                    for a in range(3):
                        u1 = work.tile([128, W], f32, tag="u1",
                                       name=f"u1{a}")
                        nc.vector.scalar_tensor_tensor(u1[:], da[a][:], 0.5,
                                                       da[a][:], ALU.is_gt,
                                                       ALU.subtract)
                        nc.vector.scalar_tensor_tensor(da[a][:], da[a][:],
                                                       -0.5, u1[:],
                                                       ALU.is_lt,
                                                       ALU.subtract)
                    terms = [(0, 0, 0), (1, 1, 1), (2, 2, 2),
                             (0, 1, 3), (0, 2, 4), (1, 2, 5)]
                    acc = gpool.tile([128, W], f32, tag="acc")
                    accb = gpool.tile([128, W], f32, tag="accb")
                    cur, nxt = acc, accb
                    for i, (ia, ib, gi) in enumerate(terms):
                        pr = work.tile([128, W], f32, tag="pr",
                                       name=f"pr{i}")
                        nc.vector.tensor_tensor(pr[:], da[ia][:], da[ib][:],
                                                ALU.mult)
                        if i == 0:
                            nc.vector.tensor_scalar_mul(cur[:], pr[:],
                                                        gcol[:, 0:1])
                        else:
                            nc.vector.scalar_tensor_tensor(
                                nxt[:], pr[:], gcol[:, gi:gi + 1], cur[:],
                                ALU.mult, ALU.add)
                            cur, nxt = nxt, cur
                    # gcol holds -G entries (cur = -d^2); d12 = sqrt(-cur)
                    nc.vector.tensor_scalar_min(cur[:], cur[:], -1e-12)
                    nc.scalar.activation(cur[:], cur[:], AF.Ln, scale=-1.0)
                    nc.scalar.activation(d12[:], cur[:], AF.Exp, scale=0.5)
                    if debug:
                        nc.sync.dma_start(dbg["d12"].ap(), d12[:])

                    # d12 -> DRAM slot-major -> broadcast -> gaussians
                    nc.sync.dma_start(dflat2[:].transpose([1, 0]), d12[:])
                    dfb = (dflat2[:].rearrange("c p -> (c p)").unsqueeze(0)
                           .to_broadcast([KG, NB * M * 128]))
                    gin = gpool.tile([KG, NB * M * 128], f32, tag="gin")
                    nc.sync.dma_start(gin[:], dfb)
                    nc.scalar.activation(gin[:], gin[:], AF.Square,
                                         bias=noff[:])
                    nc.scalar.activation(gss[:], gin[:], AF.Exp,
                                         scale=COEFF)
                    if debug:
                        gdbg = dbp.tile([KG, 512], f32, tag="gdbg")
                        nc.vector.tensor_copy(gdbg[:], gss[:, 0:512])
                        nc.sync.dma_start(dbg["gauss"].ap(), gdbg[:])

                # ================= stage C: conv layers ===================
                def softplus_ln(out_ap, in_ap, pool, shape, tag, dtyp):
                    """out = relu(x) + ln(1 + exp(-|x|)); ACT: Abs,Exp,Ln."""
                    t = pool.tile(shape, dtyp, tag="sptmp",
                                  name=tag + "_t")
                    nc.scalar.activation(t[:], in_ap, AF.Abs)
                    nc.scalar.activation(t[:], t[:], AF.Exp, scale=-1.0)
                    nc.scalar.activation(t[:], t[:], AF.Ln, bias=1.0)
                    nc.vector.scalar_tensor_tensor(out_ap, in_ap, 0.0, t[:],
                                                   ALU.max, ALU.add)

                # per b: psum zz[p,(m,256)] = sum_m gauss_m @ [wn1|wn2]
                #        + ident @ hjWg_b (both layers' neighbor term)
                # evac once to bf16; per layer: +hi, LN, sigmoid*softplus
                # (all ACT ops live in the exp/ln table set).
                gss_v = gss[:].rearrange("k (b m a) -> k b m a", b=NB, m=M)

                with tc.tile_pool(name="psCz", bufs=1, space="PSUM") as psCz, \
                     tc.tile_pool(name="psCg", bufs=1, space="PSUM") as psCg:
                    zt = [None] * NB
                    for b in range(NB):
                        zz = psCz.tile([128, M * 4 * FEA], f32, tag="zz")
                        for m in range(M):
                            nc.tensor.matmul(
                                zz[:, m * 256:(m + 1) * 256],
                                gss_v[:, b, m, :], wnx[:],
                                start=True, stop=False)
                            nc.tensor.matmul(
                                zz[:, m * 256:(m + 1) * 256], identb[:],
                                hjwg[:, (b * M + m) * 256:
                                     (b * M + m + 1) * 256],
                                start=False, stop=True)
                        zt[b] = cvp.tile([128, M * 4 * FEA], bf16,
                                         tag=f"zt{b}", name=f"zt{b}")
                        nc.scalar.activation(zt[b][:], zz[:], AF.Copy)

                    for L in range(2):
                        hi = hi1 if L == 0 else hi2
                        aprev = atom0 if L == 0 else atom1
                        anext = atom1 if L == 0 else atom2
                        xm = [None] * NB
                        lt = [None] * NB
                        ug = [None] * NB
                        lv = [None] * NB
                        rsd = [None] * NB
                        att = [None] * NB
                        spa = [None] * NB
                        # phase 1 (DVE only): t=z+hi, mu, xm, sq->lt, vv->lv
                        for b in range(NB):
                            if L == 0:
                                t = tL1[b]
                            else:
                                t = cvp.tile([128, M * 128], bf16,
                                             tag="tcs", name=f"tc{L}{b}")
                                nc.vector.tensor_tensor(
                                    t[:].rearrange("p (m f) -> p m f", m=M),
                                    ztB[b][:]
                                    .rearrange("p (m f) -> p m f", m=M),
                                    hi[b][:].unsqueeze(1)
                                    .to_broadcast([128, M, 128]), ALU.add)
                            tv = t[:].rearrange("p (m f) -> p m f", m=M)
                            if debug and L == 0 and b == 0:
                                for zc in range(2):
                                    zdbg = dbp.tile([128, M * 64], f32,
                                                    tag="zdbg",
                                                    name=f"zdbg{zc}")
                                    nc.vector.tensor_copy(
                                        zdbg[:],
                                        t[:, zc * M * 64:(zc + 1) * M * 64])
                                    nc.sync.dma_start(
                                        dbg["z1"].ap()
                                        [:, zc * M * 64:(zc + 1) * M * 64],
                                        zdbg[:])
                            mu = work.tile([128, M], bf16, tag="mu")
                            nc.vector.tensor_reduce(mu[:], tv, axis=AX.X,
                                                    op=ALU.add)
                            xm[b] = cvp.tile([128, M * 128], bf16,
                                             tag=f"xm{b}", name=f"xm{L}{b}")
                            xv = xm[b][:].rearrange("p (m f) -> p m f", m=M)
                            nc.vector.scalar_tensor_tensor(
                                xv,
                                mu[:].unsqueeze(2)
                                .to_broadcast([128, M, 128]),
                                -1.0 / 128.0, tv, ALU.mult, ALU.add)
                            # lt[b] doubles as the x^2 scratch before Abs
                            lt[b] = cvp.tile([128, M * 128], bf16,
                                             tag=f"lt{b}", name=f"lt{L}{b}")
                            nc.vector.tensor_tensor(lt[b][:], xm[b][:],
                                                    xm[b][:], ALU.mult)
                            vv = work.tile([128, M], bf16, tag="vv")
                            nc.vector.tensor_reduce(
                                vv[:],
                                lt[b][:].rearrange("p (m f) -> p m f", m=M),
                                axis=AX.X, op=ALU.add)
                            lv[b] = cvp.tile([128, M], f32, tag=f"lv{b}",
                                             name=f"lv{L}{b}")
                            nc.vector.tensor_copy(lv[b][:], vv[:])
                        # phase 2 (ACT batched): rsd = exp(-0.5 ln(v+eps))
                        for b in range(NB):
                            nc.scalar.activation(lv[b][:], lv[b][:], AF.Ln,
                                                 scale=1.0 / 128.0,
                                                 bias=epsc[:])
                        for b in range(NB):
                            rsd[b] = cvp.tile([128, M], bf16,
                                              tag=f"rsd{b}",
                                              name=f"rsd{L}{b}")
                            nc.scalar.activation(rsd[b][:], lv[b][:],
                                                 AF.Exp, scale=-0.5)
                        # phase 3 (DVE): normalize
                        for b in range(NB):
                            xv = xm[b][:].rearrange("p (m f) -> p m f", m=M)
                            nc.vector.tensor_tensor(
                                xv, xv,
                                rsd[b][:].unsqueeze(2)
                                .to_broadcast([128, M, 128]), ALU.mult)
                        # phase 4 (ACT batched): l = ln(1+exp(-|x|))
                        for b in range(NB):
                            nc.scalar.activation(lt[b][:], xm[b][:], AF.Abs)
                        for b in range(NB):
                            nc.scalar.activation(lt[b][:], lt[b][:], AF.Exp,
                                                 scale=-1.0)
                        for b in range(NB):
                            nc.scalar.activation(lt[b][:], lt[b][:], AF.Ln,
                                                 bias=1.0)
                        # phase 5: sig = exp(min(f,0)-l_f); sp = relu(c)+l_c
                        for b in range(NB):
                            xv = xm[b][:].rearrange("p (m f) -> p m f", m=M)
                            lv_ = lt[b][:].rearrange("p (m f) -> p m f",
                                                     m=M)
                            ug[b] = cvp.tile([128, M * FEA], bf16,
                                             tag=f"ug{b}", name=f"ug{L}{b}")
                            nc.vector.scalar_tensor_tensor(
                                ug[b][:].rearrange("p (m f) -> p m f", m=M),
                                xv[:, :, 0:FEA], 0.0,
                                lv_[:, :, 0:FEA], ALU.min, ALU.subtract)
                        for b in range(NB):
                            nc.scalar.activation(ug[b][:], ug[b][:], AF.Exp)
                        for b in range(NB):
                            xv = xm[b][:].rearrange("p (m f) -> p m f", m=M)
                            lv_ = lt[b][:].rearrange("p (m f) -> p m f",
                                                     m=M)
                            sp = work.tile([128, M * FEA], bf16, tag="sps")
                            spv = sp[:].rearrange("p (m f) -> p m f", m=M)
                            nc.vector.scalar_tensor_tensor(
                                spv, xv[:, :, FEA:128], 0.0,
                                lv_[:, :, FEA:128], ALU.max, ALU.add)
                            nc.vector.tensor_tensor(ug[b][:], ug[b][:],
                                                    sp[:], ALU.mult)
                            u_ = ug[b]
                            ns = work.tile([128, FEA], f32, tag="ns")
                            nc.vector.tensor_tensor(
                                u_[:, 0:6 * FEA], u_[:, 0:6 * FEA],
                                u_[:, 6 * FEA:12 * FEA], ALU.add)
                            nc.vector.tensor_tensor(
                                u_[:, 0:3 * FEA], u_[:, 0:3 * FEA],
                                u_[:, 3 * FEA:6 * FEA], ALU.add)
                            nc.vector.tensor_tensor(
                                u_[:, 0:FEA], u_[:, 0:FEA],
                                u_[:, FEA:2 * FEA], ALU.add)
                            nc.vector.tensor_tensor(
                                ns[:], u_[:, 0:FEA],
                                u_[:, 2 * FEA:3 * FEA], ALU.add)
                            att[b] = cvp.tile([128, FEA], f32,
                                              tag=f"at{b}",
                                              name=f"at{L}{b}")
                            nc.vector.tensor_tensor(att[b][:], aprev[b][:],
                                                    ns[:], ALU.add)
                        # phase 6 (ACT batched): atom softplus
                        for b in range(NB):
                            spa[b] = cvp.tile([128, FEA], f32,
                                              tag=f"spa{b}",
                                              name=f"spa{L}{b}")
                            nc.scalar.activation(spa[b][:], att[b][:],
                                                 AF.Abs)
                        for b in range(NB):
                            nc.scalar.activation(spa[b][:], spa[b][:],
                                                 AF.Exp, scale=-1.0)
                        for b in range(NB):
                            nc.scalar.activation(spa[b][:], spa[b][:],
                                                 AF.Ln, bias=1.0)
                        for b in range(NB):
                            nc.vector.scalar_tensor_tensor(
                                anext[b][:], att[b][:], 0.0, spa[b][:],
                                ALU.max, ALU.add)
                        if L == 0:
                            # hi2 from atom1 (ones row -> exact bias fold)
                            a1x = epool.tile([FEA + 1, NL], f32, tag="a1x")
                            nc.vector.memset(a1x[FEA:FEA + 1, :], 1.0)
                            for b in range(NB):
                                sl = slice(b * 128, (b + 1) * 128)
                                tp = psCg.tile([FEA, 128], f32, tag="tpC")
                                nc.tensor.transpose(tp[:], atom1[b][:],
                                                    ident[:])
                                nc.scalar.activation(a1x[0:FEA, sl], tp[:],
                                                     AF.Copy)
                                hp = psCg.tile([128, 2 * FEA], f32,
                                               tag="hpC")
                                nc.tensor.matmul(hp[:], a1x[:, sl], wib2[:],
                                                 start=True, stop=True)
                                nc.vector.tensor_copy(hi2[b][:], hp[:])
                            if debug:
                                for b in range(NB):
                                    nc.sync.dma_start(
                                        dbg["atom1"].ap()
                                        [b * 128:(b + 1) * 128, :],
                                        atom1[b][:])

                for b in range(NB):
                    nc.sync.dma_start(d_out.ap()[b * 128:(b + 1) * 128, :],
                                      atom2[b][:])

    _body()
    nc.compile()
    return nc


def _prep_inputs(inputs):
    """Host-side layout prep. Returns (in_maps, host_ctx)."""
    import ml_dtypes
    bf = ml_dtypes.bfloat16
    f32 = np.float32
    lat = np.asarray(inputs["lat_pred"], f32)
    fr = np.ascontiguousarray(np.asarray(inputs["fracs_pred"], f32))
    sl = np.ascontiguousarray(np.asarray(inputs["species_logits"], f32))
    occ = np.asarray(inputs["occ_logits"], f32)
    emb_w = np.asarray(inputs["emb_w"], f32)
    emb_b = np.asarray(inputs["emb_b"], f32)
    w1 = np.asarray(inputs["w1"], f32); b1 = np.asarray(inputs["b1"], f32)
    w2 = np.asarray(inputs["w2"], f32); b2 = np.asarray(inputs["b2"], f32)

    G = (lat.astype(np.float64) @ lat.T.astype(np.float64))
    wroot = np.sqrt(np.diag(G)).astype(f32)

    frecs = np.zeros((N, 64), f32)
    frecs[:, 0:3] = fr

    gneg = (-np.array([G[0, 0], G[1, 1], G[2, 2],
                       2 * G[0, 1], 2 * G[0, 2], 2 * G[1, 2]])).astype(f32)

    splogT = np.zeros((128, N), f32)
    splogT[0:SPECIES, :] = sl.T

    embwx = np.concatenate([emb_w, np.ones((SPECIES, 1), f32)], 1)
    # hi1 path: psum = af_un@wi1 + rs*(b1 + emb_b@wi1); * (1/rs) gives
    # (af_un/rs + emb_b)@wi1 + b1 = af@wi1 + b1 exactly.
    wib1 = np.ascontiguousarray(np.concatenate(
        [w1[0:FEA, :], (b1 + emb_b @ w1[0:FEA, :])[None, :]], 0))
    wib2 = np.ascontiguousarray(
        np.concatenate([w2[0:FEA, :], b2[None, :]], 0))
    wjx = np.ascontiguousarray(
        np.concatenate([w1[FEA:2 * FEA, :], w2[FEA:2 * FEA, :]], 1)).astype(bf)
    wnx = np.ascontiguousarray(
        np.concatenate([w1[2 * FEA:, :], w2[2 * FEA:, :]], 1)).astype(bf)

    shared = dict(
        splogT=splogT,
        fracsT=np.ascontiguousarray(fr.T),
        frecs=frecs,
        embwx=np.ascontiguousarray(embwx),
        embbrow=np.ascontiguousarray(np.broadcast_to(emb_b, (128, FEA))),
        wib1=wib1, wib2=wib2, wjx=wjx, wnx=wnx,
        gcol=np.ascontiguousarray(np.broadcast_to(gneg, (128, 6))),
        wroot=wroot.reshape(3, 1),
        noff=(-OFFSET).reshape(KG, 1),
        blockoff=np.ascontiguousarray(np.broadcast_to(
            np.repeat(np.arange(NBLK, dtype=np.uint32) * BLK, 8),
            (128, CAND))).astype(np.uint32),
        identb=np.eye(128, dtype=f32).astype(bf),
        ident=np.eye(128, dtype=f32),
    )
    in_maps = []
    for c in range(NCORES):
        rows = slice(c * NL, (c + 1) * NL)
        selfid = (c * NL + np.arange(128, dtype=f32)[:, None]
                  + 128 * np.arange(NB, dtype=f32)[None, :]).astype(f32)
        sploclT = np.zeros((128, NL), f32)
        sploclT[0:SPECIES, :] = sl[rows].T
        m = dict(shared)
        m.update(sploclT=sploclT, fl=np.ascontiguousarray(fr[rows]),
                 flT=np.ascontiguousarray(fr[rows].T),
                 selfid=np.ascontiguousarray(selfid))
        in_maps.append(m)
    host = dict(occ=occ, fc_w=np.asarray(inputs["fc_w"], f32),
                fc_b=np.asarray(inputs["fc_b"], f32))
    return in_maps, host


def _host_finish(results, host):
    a2 = np.concatenate([np.asarray(r["atom2"]) for r in results], 0)
    occp = 1.0 / (1.0 + np.exp(-host["occ"].astype(np.float64)))
    graph = (a2.astype(np.float64) * occp[:, None]).sum(0) / (occp.sum()
                                                              + 1e-6)
    out = graph @ host["fc_w"].astype(np.float64) + host["fc_b"]
    return out.astype(np.float32)


def kernel(**inputs) -> np.ndarray:
    from concourse import bass_utils

    in_maps, host = _prep_inputs(inputs)
    key = "prog"
    if key not in _cache:
        _cache[key] = _build_program(debug=False)
    nc = _cache[key]
    res = bass_utils.run_bass_kernel_spmd(nc, in_maps,
                                          core_ids=list(range(NCORES)))
    return _host_finish(res.results, host)


# revision 41
# speedup vs baseline: 1.0182x; 1.0182x over previous
"""Trainium2 Bass kernel for nn_DifferentiableCGCNN (N=4096 atoms, 8 NeuronCores).

v2 strategy (SPMD, one identical program per core, no collectives):
  - neighbor SELECTION by surrogate rank only (validated offline: end-to-end
    output shift 5e-4 << 2e-2 tol): torus surrogate dots as bf16 PE matmuls,
    top-8 per 512-block (max/max_index), then top-12 of the 64 candidates
    by value; global indices via compare-select masks.
  - exact distances only for the selected 12 (1.5MB gather of padded fracs
    rows), d12 = sqrt(e^T G e) on DVE/ACT.
  - embedding: host-transposed species logits -> Exp -> row matmuls with
    [embw|ones] so the softmax sum rides along as column 64; normalization
    folded into the PSUM evacuation (x * 1/sum + bias). bf16 atom-feature
    table [N,128] written to DRAM for the neighbor gather. Local shard is
    handled feature-major from a per-core sploclT input; the sum row rides
    as partition 64 so the bias folds exactly ((af_un@W + rs*b) * 1/rs).
  - neighbor features: a premultiplied hjW = af @ [wj1|wj2] table (bf16,
    512B rows) is gathered atom-major with plain dma_gather (1024-idx
    chunks; transpose-mode gather and >1024-descriptor gathers CRASH on
    real HW) and added into PSUM via one wide identity matmul; gaussians
    are built feature-major and contracted with [wn1|wn2] per (b,m).
  - LN classic (reduce + fused mean-subtract); sigmoid and softplus share
    one Abs/Exp/Ln pass (sig = exp(min(f,0)-l_f), sp = relu(c)+l_c) so all
    stage-C ACT ops use exp/ln-capable tables; ops are batched per phase
    across the four row tiles because the act-table chooser is greedy and
    an Exp<->Ln ping-pong costs 1.28us per switch. Elementwise in bf16
    for the DVE 2x mode; stage E's Exp carries a zero bias data-dependent
    on the Sin outputs to keep the sin/exp tables from thrashing.
  - final occupancy pooling + fc on host from per-core [512,64] outputs.
"""
import os
import sys

import numpy as np

for _p in ("/opt/trn_rl_repo",):
    if os.path.isdir(_p) and _p not in sys.path:
        sys.path.insert(0, _p)

N = 4096
NCORES = 8
NL = N // NCORES          # 512 atoms per core
NB = NL // 128            # 4 row tiles per core
SPECIES = 100
FEA = 64
KG = 64                   # gaussian filters
M = 12                    # neighbors
BLK = 512                 # surrogate block width
NBLK = N // BLK           # 8 blocks
CAND = NBLK * 8           # 64 candidates per row
LN_EPS = 1e-5
BIG = 1e30

OFFSET = np.linspace(0.0, 8.0, KG).astype(np.float32)
COEFF = float(-0.5 / (8.0 / (KG - 1)) ** 2)

_cache = {}


def _build_program(debug: bool = False):
    from contextlib import ExitStack

    import concourse.bacc as bacc
    import concourse.mybir as mybir
    from concourse.tile import TileContext

    dt = mybir.dt
    AF = mybir.ActivationFunctionType
    ALU = mybir.AluOpType
    AX = mybir.AxisListType
    f32 = dt.float32
    bf16 = dt.bfloat16
    f32r = dt.float32r

    nc = bacc.Bacc("TRN2", target_bir_lowering=False, debug=False,
                   enable_asserts=False)

    # ---- dram inputs ----
    d_splogT = nc.dram_tensor("splogT", [128, N], f32, kind="ExternalInput")
    d_sploclT = nc.dram_tensor("sploclT", [128, NL], f32,
                               kind="ExternalInput")
    d_fracsT = nc.dram_tensor("fracsT", [3, N], f32, kind="ExternalInput")
    d_fl = nc.dram_tensor("fl", [NL, 3], f32, kind="ExternalInput")
    d_flT = nc.dram_tensor("flT", [3, NL], f32, kind="ExternalInput")
    d_frecs = nc.dram_tensor("frecs", [N, 64], f32, kind="ExternalInput")
    d_embwx = nc.dram_tensor("embwx", [SPECIES, FEA + 1], f32,
                             kind="ExternalInput")
    d_embbrow = nc.dram_tensor("embbrow", [128, FEA], f32,
                               kind="ExternalInput")
    d_wib1 = nc.dram_tensor("wib1", [FEA + 1, 2 * FEA], f32,
                            kind="ExternalInput")
    d_wib2 = nc.dram_tensor("wib2", [FEA + 1, 2 * FEA], f32,
                            kind="ExternalInput")
    d_wjx = nc.dram_tensor("wjx", [FEA, 4 * FEA], bf16, kind="ExternalInput")
    d_wnx = nc.dram_tensor("wnx", [FEA, 4 * FEA], bf16, kind="ExternalInput")
    d_gcol = nc.dram_tensor("gcol", [128, 6], f32, kind="ExternalInput")
    d_wroot = nc.dram_tensor("wroot", [3, 1], f32, kind="ExternalInput")
    d_noff = nc.dram_tensor("noff", [KG, 1], f32, kind="ExternalInput")
    d_selfid = nc.dram_tensor("selfid", [128, NB], f32, kind="ExternalInput")
    d_blockoff = nc.dram_tensor("blockoff", [128, CAND], dt.uint32,
                                kind="ExternalInput")
    d_identb = nc.dram_tensor("identb", [128, 128], bf16, kind="ExternalInput")
    d_ident = nc.dram_tensor("ident", [128, 128], f32, kind="ExternalInput")

    d_out = nc.dram_tensor("atom2", [NL, FEA], f32, kind="ExternalOutput")

    dbg = {}
    if debug:
        def dbg_t(name, shape, dtyp=f32):
            dbg[name] = nc.dram_tensor("dbg_" + name, shape, dtyp,
                                       kind="ExternalOutput")
        dbg_t("mxall", [128, NB * CAND])
        dbg_t("nidx", [128, NB * M], dt.uint32)
        dbg_t("d12", [128, NB * M])
        dbg_t("af0", [128, FEA])
        dbg_t("hi1", [128, 2 * FEA])
        dbg_t("hjg", [128, 512])
        dbg_t("gauss", [KG, 512])
        dbg_t("z1", [128, M * 128])
        dbg_t("atom1", [NL, FEA])

    def _body():  # noqa: PLR0915
        with TileContext(nc) as tc, \
             nc.allow_low_precision(reason="bf16 staging is intentional"):
            with ExitStack() as ctx:
                const = ctx.enter_context(tc.tile_pool(name="const", bufs=1))
                persist = ctx.enter_context(tc.tile_pool(name="persist",
                                                         bufs=1))
                work = ctx.enter_context(tc.tile_pool(name="work", bufs=4))
                spool = ctx.enter_context(tc.tile_pool(name="spool", bufs=1))
                epool = ctx.enter_context(tc.tile_pool(name="epool", bufs=1))
                ework = ctx.enter_context(tc.tile_pool(name="ework", bufs=4))
                gpool = ctx.enter_context(tc.tile_pool(name="gpool", bufs=1))
                cvp = ctx.enter_context(tc.tile_pool(name="cvp", bufs=1))
                dbp = ctx.enter_context(tc.tile_pool(name="dbp", bufs=1))
                drp = ctx.enter_context(tc.tile_pool(name="dram", bufs=1,
                                                     space="DRAM"))

                def cload(dram, shape, dtyp=f32):
                    t = const.tile(shape, dtyp, tag=dram.name + "_c")
                    nc.sync.dma_start(t[:], dram.ap())
                    return t

                embwx = cload(d_embwx, [SPECIES, FEA + 1])
                embbrow = cload(d_embbrow, [128, FEA])
                wib1 = cload(d_wib1, [FEA + 1, 2 * FEA])
                wib2 = cload(d_wib2, [FEA + 1, 2 * FEA])
                wjx = cload(d_wjx, [FEA, 4 * FEA], bf16)
                wnx = cload(d_wnx, [FEA, 4 * FEA], bf16)
                gcol = cload(d_gcol, [128, 6])
                wroot = cload(d_wroot, [3, 1])
                gsq = const.tile([3, 1], f32, tag="gsq")
                nc.vector.tensor_tensor(gsq[:], wroot[:], wroot[:], ALU.mult)
                noff = cload(d_noff, [KG, 1])
                selfid = cload(d_selfid, [128, NB])
                blockoff = cload(d_blockoff, [128, CAND], dt.uint32)
                identb = cload(d_identb, [128, 128], bf16)
                ident = cload(d_ident, [128, 128])
                mpi = const.tile([3, 1], f32, tag="mpi")
                nc.vector.memset(mpi[:], -np.pi)
                hpi = const.tile([3, 1], f32, tag="hpi")
                nc.vector.memset(hpi[:], np.pi / 2)
                epsc = const.tile([128, 1], f32, tag="epsc")
                nc.vector.memset(epsc[:], LN_EPS)

                # dram scratch
                hjw = drp.tile([N, 4 * FEA], bf16, tag="hjw")
                dflat2 = drp.tile([NB * M, 128], f32, tag="dflat2")
                hbmI = drp.tile([16, NB * M * 8], dt.int16, tag="hbmI")

                # persistent cross-stage tiles
                nidx16 = persist.tile([128, NB * M], dt.int16, tag="nidx16")
                d12 = persist.tile([128, NB * M], f32, tag="d12")
                hjwg = persist.tile([128, NB * M * 4 * FEA], bf16,
                                    tag="hjwg")
                gss = persist.tile([KG, NB * M * 128], bf16, tag="gss")
                flb = [persist.tile([128, 3], f32, tag=f"flb_{b}",
                                    name=f"flb_{b}") for b in range(NB)]
                rrloc = [persist.tile([128, 1], f32, tag=f"rr_{b}",
                                      name=f"rr_{b}") for b in range(NB)]
                atom0 = [persist.tile([128, FEA], f32, tag=f"a0_{b}",
                                      name=f"a0_{b}") for b in range(NB)]
                atom1 = [persist.tile([128, FEA], f32, tag=f"a1_{b}",
                                      name=f"a1_{b}") for b in range(NB)]
                atom2 = [persist.tile([128, FEA], f32, tag=f"a2_{b}",
                                      name=f"a2_{b}") for b in range(NB)]
                hi1 = [persist.tile([128, 2 * FEA], bf16, tag=f"hi1_{b}",
                                    name=f"hi1_{b}") for b in range(NB)]
                hi2 = [persist.tile([128, 2 * FEA], bf16, tag=f"hi2_{b}",
                                    name=f"hi2_{b}") for b in range(NB)]

                # ======== stages S (select) + E (embed) + G (gather) ======
                uT = spool.tile([36, N], bf16, tag="uT")
                uloc = spool.tile([36, NL], bf16, tag="uloc")
                fT = spool.tile([3, N], f32, tag="fT")
                fTl = spool.tile([3, NL], f32, tag="fTl")
                mxall = spool.tile([128, NB * CAND], f32, tag="mxall")
                idxgF = spool.tile([128, NB * CAND], f32, tag="idxgF")
                idxg = spool.tile([128, NB * CAND], dt.uint32, tag="idxg")
                vals = spool.tile([128, NB * 16], f32, tag="vals")
                nidxF = spool.tile([128, NB * M], f32, tag="nidxF")
                msk = spool.tile([128, NB * M * CAND], f32, tag="msk")

                expT = epool.tile([128, N], f32, tag="expT")
                explT = epool.tile([128, NL], f32, tag="explT")
                afTx = epool.tile([FEA + 1, NL], f32, tag="afTx")

                with tc.tile_pool(name="psS", bufs=2, space="PSUM") as psS, \
                     tc.tile_pool(name="psA", bufs=1, space="PSUM") as psA, \
                     tc.tile_pool(name="psL", bufs=1, space="PSUM") as psL, \
                     tc.tile_pool(name="psT", bufs=1, space="PSUM") as psT:

                    # ---- stage S ----
                    nc.sync.dma_start(fT[:], d_fracsT.ap())
                    nc.sync.dma_start(fTl[:], d_flT.ap())
                    for b in range(NB):
                        nc.sync.dma_start(flb[b][:],
                                          d_fl.ap()[b * 128:(b + 1) * 128, :])

                    # u rows: cos at partitions 0-2, sin at 32-34. Only
                    # the local (stationary) side carries the G_aa scaling
                    # so the [*, N]-wide scaling ops disappear.
                    nc.vector.memset(uT[:], 0.0)
                    nc.scalar.activation(uT[32:35, :], fT[:], AF.Sin,
                                         scale=2 * np.pi, bias=mpi[:])
                    nc.scalar.activation(fT[:], fT[:], AF.Abs,
                                         scale=2 * np.pi, bias=mpi[:])
                    nc.scalar.activation(uT[0:3, :], fT[:], AF.Sin,
                                         scale=-1.0, bias=hpi[:])
                    ulf = spool.tile([36, NL], f32, tag="ulf")
                    nc.vector.memset(ulf[:], 0.0)
                    nc.scalar.activation(ulf[32:35, :], fTl[:], AF.Sin,
                                         scale=2 * np.pi, bias=mpi[:])
                    nc.scalar.activation(fTl[:], fTl[:], AF.Abs,
                                         scale=2 * np.pi, bias=mpi[:])
                    nc.scalar.activation(ulf[0:3, :], fTl[:], AF.Sin,
                                         scale=-1.0, bias=hpi[:])
                    nc.vector.tensor_scalar_mul(ulf[0:3, :], ulf[0:3, :],
                                                gsq[:])
                    nc.vector.tensor_scalar_mul(ulf[32:35, :],
                                                ulf[32:35, :], gsq[:])
                    nc.vector.tensor_copy(uloc[:], ulf[:])

                    idxg_v = idxg[:].rearrange("p (b c) -> p b c", b=NB)
                    mx_v = mxall[:].rearrange("p (b c) -> p b c", b=NB)

                    for b in range(NB):
                        for j in range(8):
                            dps = psS.tile([128, 512], f32, tag="dots")
                            nc.tensor.matmul(
                                dps[:], uloc[:, b * 128:(b + 1) * 128],
                                uT[:, j * 512:(j + 1) * 512],
                                start=True, stop=True)
                            nc.vector.max(
                                out=mx_v[:, b, j * 8:(j + 1) * 8], in_=dps[:])
                            nc.vector.max_index(
                                idxg_v[:, b, j * 8:(j + 1) * 8],
                                mx_v[:, b, j * 8:(j + 1) * 8], dps[:])

                    nc.vector.tensor_tensor(
                        idxg_v, idxg_v,
                        blockoff[:].unsqueeze(1).to_broadcast([128, NB, CAND]),
                        ALU.add)
                    nc.vector.tensor_copy(idxgF[:], idxg[:])

                    # self-exclusion on candidate values
                    smsk = work.tile([128, NB * CAND], f32, tag="smsk")
                    nc.vector.tensor_tensor(
                        smsk[:].rearrange("p (b c) -> p b c", b=NB),
                        idxgF[:].rearrange("p (b c) -> p b c", b=NB),
                        selfid[:].unsqueeze(2).to_broadcast([128, NB, CAND]),
                        ALU.is_equal)
                    nc.vector.scalar_tensor_tensor(mxall[:], smsk[:], -BIG,
                                                   mxall[:], ALU.mult,
                                                   ALU.add)
                    if debug:
                        nc.sync.dma_start(dbg["mxall"].ap(), mxall[:])

                    # top-12 of 64 per tile by value
                    for b in range(NB):
                        seg = mxall[:, b * CAND:(b + 1) * CAND]
                        nc.vector.max(out=vals[:, b * 16:b * 16 + 8], in_=seg)
                        mr = work.tile([128, CAND], f32, tag="mr")
                        nc.vector.match_replace(
                            out=mr[:],
                            in_to_replace=vals[:, b * 16:b * 16 + 8],
                            in_values=seg, imm_value=-BIG)
                        nc.vector.max(out=vals[:, b * 16 + 8:b * 16 + 16],
                                      in_=mr[:])

                    # global indices of the selected 12 via compare-select
                    vals_v = vals[:].rearrange("p (b v) -> p b v", b=NB)
                    msk_v = msk[:].rearrange("p (b m c) -> p b m c", b=NB,
                                             m=M)
                    nc.vector.tensor_tensor(
                        msk_v,
                        mxall[:].rearrange("p (b c) -> p b c", b=NB)
                        .unsqueeze(2).to_broadcast([128, NB, M, CAND]),
                        vals_v[:, :, 0:M].unsqueeze(3)
                        .to_broadcast([128, NB, M, CAND]),
                        ALU.is_equal)
                    nc.vector.tensor_tensor(
                        msk_v, msk_v,
                        idxgF[:].rearrange("p (b c) -> p b c", b=NB)
                        .unsqueeze(2).to_broadcast([128, NB, M, CAND]),
                        ALU.mult)
                    nc.vector.tensor_reduce(
                        nidxF[:].rearrange("p (b m) -> p b m", b=NB), msk_v,
                        axis=AX.X, op=ALU.max)
                    nc.vector.tensor_copy(nidx16[:], nidxF[:])
                    if debug:
                        nidxU = dbp.tile([128, NB * M], dt.uint32,
                                          tag="nidxU")
                        nc.vector.tensor_copy(nidxU[:], nidxF[:])
                        nc.sync.dma_start(dbg["nidx"].ap(), nidxU[:])

                    # ---- stage E (independent of S; overlaps it) ----
                    for h in range(8):
                        sl = slice(h * 512, (h + 1) * 512)
                        nc.sync.dma_start(expT[:, sl], d_splogT.ap()[:, sl])
                        nc.scalar.activation(expT[0:SPECIES, sl],
                                             expT[0:SPECIES, sl], AF.Exp)
                    nc.sync.dma_start(explT[:], d_sploclT.ap())
                    nc.scalar.activation(explT[0:SPECIES, :],
                                         explT[0:SPECIES, :], AF.Exp)

                    # full-graph neighbor table: hjW rows =
                    # af_row @ [wj1|wj2]  (af normalized + emb-biased)
                    for c in range(N // 128):
                        ps = psA.tile([128, FEA + 1], f32, tag="psA")
                        nc.tensor.matmul(
                            ps[:], expT[0:SPECIES, c * 128:(c + 1) * 128],
                            embwx[:], start=True, stop=True)
                        rr = ework.tile([128, 1], f32, tag="rr")
                        nc.vector.reciprocal(rr[:], ps[:, FEA:FEA + 1])
                        ab = ework.tile([128, FEA], bf16, tag="ab")
                        nc.vector.scalar_tensor_tensor(
                            ab[:], ps[:, 0:FEA], rr[:], embbrow[:],
                            ALU.mult, ALU.add)
                        tp2 = psA.tile([FEA, 128], bf16, tag="tp2")
                        nc.tensor.transpose(tp2[:], ab[:], identb[:])
                        abT = ework.tile([FEA, 128], bf16, tag="abT")
                        nc.vector.tensor_copy(abT[:], tp2[:])
                        psW = psA.tile([128, 4 * FEA], f32, tag="psW")
                        nc.tensor.matmul(psW[:], abT[:], wjx[:],
                                         start=True, stop=True)
                        hw_ = ework.tile([128, 4 * FEA], bf16, tag="hw_")
                        nc.vector.tensor_copy(hw_[:], psW[:])
                        nc.sync.dma_start(hjw[c * 128:(c + 1) * 128, :],
                                          hw_[:])

                    # local feature-major [af_unnorm ; sums] via f32r matmuls
                    psl = psL.tile([FEA + 1, NL], f32, tag="psl")
                    nc.tensor.matmul(psl[0:FEA, :],
                                     embwx[:, 0:FEA],
                                     explT[0:SPECIES, :],
                                     start=True, stop=True)
                    nc.tensor.matmul(psl[FEA:FEA + 1, :],
                                     embwx[:, FEA:FEA + 1],
                                     explT[0:SPECIES, :],
                                     start=True, stop=True)
                    nc.scalar.activation(afTx[:], psl[:], AF.Copy)

                    # per-tile: rows (atom0, rr) via PE transpose; hi1 matmul
                    for b in range(NB):
                        sl = slice(b * 128, (b + 1) * 128)
                        tp = psT.tile([128, FEA + 1], f32, tag="tpE")
                        nc.tensor.transpose(tp[:], afTx[:, sl],
                                            ident[0:FEA + 1, 0:FEA + 1])
                        nc.vector.reciprocal(rrloc[b][:], tp[:, FEA:FEA + 1])
                        nc.vector.scalar_tensor_tensor(
                            atom0[b][:], tp[:, 0:FEA], rrloc[b][:],
                            embbrow[:], ALU.mult, ALU.add)
                        hp = psT.tile([128, 2 * FEA], f32, tag="hpE")
                        nc.tensor.matmul(hp[:], afTx[:, sl], wib1[:],
                                         start=True, stop=True)
                        nc.vector.tensor_scalar_mul(hi1[b][:], hp[:],
                                                    rrloc[b][:])
                    if debug:
                        nc.sync.dma_start(dbg["af0"].ap(), atom0[0][:])
                        hj1f = dbp.tile([128, 2 * FEA], f32, tag="hj1f")
                        nc.vector.tensor_copy(hj1f[:], hi1[0][:])
                        nc.sync.dma_start(dbg["hi1"].ap(), hj1f[:])

                    # ---- stage G: gathers + d12 + gauss ----
                    hv = hbmI[:].rearrange("s (c e) -> s c e", e=8)
                    for w in range(8):
                        nc.sync.dma_start(hv[:, :, w],
                                          nidx16[16 * w:16 * (w + 1), :])
                    idxsG = gpool.tile([128, NB * M * 8], dt.int16,
                                       tag="idxsG")
                    for r in range(8):
                        nc.sync.dma_start(idxsG[16 * r:16 * (r + 1), :],
                                          hbmI[:])

                    # fracs gather: [128, 48, 64] f32 (256B rows)
                    crec = gpool.tile([128, NB * M * 64], f32, tag="crec")
                    crec_ch = crec[:].rearrange("p (c e) -> p c e", e=64)
                    for k in range(6):
                        nc.gpsimd.dma_gather(
                            crec_ch[:, k * 8:(k + 1) * 8, :], d_frecs.ap(),
                            idxsG[:, k * 64:(k + 1) * 64], 1024, 1024, 64)

                    # neighbor z-contribution gather (512B bf16 rows,
                    # atom-major — added to PSUM via one identity matmul)
                    hjwg_ch = hjwg[:].rearrange("p (c e) -> p c e",
                                                e=4 * FEA)
                    for k in range(6):
                        nc.gpsimd.dma_gather(
                            hjwg_ch[:, k * 8:(k + 1) * 8, :], hjw[:],
                            idxsG[:, k * 64:(k + 1) * 64],
                            1024, 1024, 4 * FEA)
                    if debug:
                        hjdbg = dbp.tile([128, 512], f32, tag="hjdbg")
                        nc.vector.tensor_copy(hjdbg[:], hjwg[:, 0:512])
                        nc.sync.dma_start(dbg["hjg"].ap(), hjdbg[:])

                    # ---- d12 (exact metric for the selected 12) ----
                    da = [gpool.tile([128, NB * M], f32, tag=f"da{a}",
                                     name=f"da{a}") for a in range(3)]
                    for b in range(NB):
                        for a in range(3):
                            nc.vector.tensor_scalar(
                                da[a][:, b * M:(b + 1) * M],
                                crec_ch[:, b * M:(b + 1) * M, a],
                                flb[b][:, a:a + 1], None, op0=ALU.subtract)
                    W = NB * M
                    for a in range(3):
                        u1 = work.tile([128, W], f32, tag="u1",
                                       name=f"u1{a}")
                        nc.vector.scalar_tensor_tensor(u1[:], da[a][:], 0.5,
                                                       da[a][:], ALU.is_gt,
                                                       ALU.subtract)
                        nc.vector.scalar_tensor_tensor(da[a][:], da[a][:],
                                                       -0.5, u1[:],
                                                       ALU.is_lt,
                                                       ALU.subtract)
                    terms = [(0, 0, 0), (1, 1, 1), (2, 2, 2),
                             (0, 1, 3), (0, 2, 4), (1, 2, 5)]
                    acc = gpool.tile([128, W], f32, tag="acc")
                    accb = gpool.tile([128, W], f32, tag="accb")
                    cur, nxt = acc, accb
                    for i, (ia, ib, gi) in enumerate(terms):
                        pr = work.tile([128, W], f32, tag="pr",
                                       name=f"pr{i}")
                        nc.vector.tensor_tensor(pr[:], da[ia][:], da[ib][:],
                                                ALU.mult)
                        if i == 0:
                            nc.vector.tensor_scalar_mul(cur[:], pr[:],
                                                        gcol[:, 0:1])
                        else:
                            nc.vector.scalar_tensor_tensor(
                                nxt[:], pr[:], gcol[:, gi:gi + 1], cur[:],
                                ALU.mult, ALU.add)
                            cur, nxt = nxt, cur
                    # gcol holds -G entries (cur = -d^2); d12 = sqrt(-cur)
                    nc.vector.tensor_scalar_min(cur[:], cur[:], -1e-12)
                    nc.scalar.activation(cur[:], cur[:], AF.Ln, scale=-1.0)
                    nc.scalar.activation(d12[:], cur[:], AF.Exp, scale=0.5)
                    if debug:
                        nc.sync.dma_start(dbg["d12"].ap(), d12[:])

                    # d12 -> DRAM slot-major -> broadcast -> gaussians
                    nc.sync.dma_start(dflat2[:].transpose([1, 0]), d12[:])
                    dfb = (dflat2[:].rearrange("c p -> (c p)").unsqueeze(0)
                           .to_broadcast([KG, NB * M * 128]))
                    gin = gpool.tile([KG, NB * M * 128], f32, tag="gin")
                    nc.sync.dma_start(gin[:], dfb)
                    nc.scalar.activation(gin[:], gin[:], AF.Square,
                                         bias=noff[:])
                    nc.scalar.activation(gss[:], gin[:], AF.Exp,
                                         scale=COEFF)
                    if debug:
                        gdbg = dbp.tile([KG, 512], f32, tag="gdbg")
                        nc.vector.tensor_copy(gdbg[:], gss[:, 0:512])
                        nc.sync.dma_start(dbg["gauss"].ap(), gdbg[:])

                # ================= stage C: conv layers ===================
                def softplus_ln(out_ap, in_ap, pool, shape, tag, dtyp):
                    """out = relu(x) + ln(1 + exp(-|x|)); ACT: Abs,Exp,Ln."""
                    t = pool.tile(shape, dtyp, tag="sptmp",
                                  name=tag + "_t")
                    nc.scalar.activation(t[:], in_ap, AF.Abs)
                    nc.scalar.activation(t[:], t[:], AF.Exp, scale=-1.0)
                    nc.scalar.activation(t[:], t[:], AF.Ln, bias=1.0)
                    nc.vector.scalar_tensor_tensor(out_ap, in_ap, 0.0, t[:],
                                                   ALU.max, ALU.add)

                # per b: psum zz[p,(m,256)] = sum_m gauss_m @ [wn1|wn2]
                #        + ident @ hjWg_b (both layers' neighbor term)
                # evac once to bf16; per layer: +hi, LN, sigmoid*softplus
                # (all ACT ops live in the exp/ln table set).
                gss_v = gss[:].rearrange("k (b m a) -> k b m a", b=NB, m=M)

                with tc.tile_pool(name="psCz", bufs=1, space="PSUM") as psCz, \
                     tc.tile_pool(name="psCg", bufs=1, space="PSUM") as psCg:
                    zt = [None] * NB
                    for b in range(NB):
                        zz = psCz.tile([128, M * 4 * FEA], f32, tag="zz")
                        for m in range(M):
                            nc.tensor.matmul(
                                zz[:, m * 256:(m + 1) * 256],
                                gss_v[:, b, m, :], wnx[:],
                                start=True, stop=False)
                            nc.tensor.matmul(
                                zz[:, m * 256:(m + 1) * 256], identb[:],
                                hjwg[:, (b * M + m) * 256:
                                     (b * M + m + 1) * 256],
                                start=False, stop=True)
                        zt[b] = cvp.tile([128, M * 4 * FEA], bf16,
                                         tag=f"zt{b}", name=f"zt{b}")
                        nc.scalar.activation(zt[b][:], zz[:], AF.Copy)

                    for L in range(2):
                        hi = hi1 if L == 0 else hi2
                        aprev = atom0 if L == 0 else atom1
                        anext = atom1 if L == 0 else atom2
                        xm = [None] * NB
                        lt = [None] * NB
                        ug = [None] * NB
                        lv = [None] * NB
                        rsd = [None] * NB
                        att = [None] * NB
                        spa = [None] * NB
                        # phase 1 (DVE only): t=z+hi, mu, xm, sq->lt, vv->lv
                        for b in range(NB):
                            if L == 0:
                                t = tL1[b]
                            else:
                                t = cvp.tile([128, M * 128], bf16,
                                             tag="tcs", name=f"tc{L}{b}")
                                nc.vector.tensor_tensor(
                                    t[:].rearrange("p (m f) -> p m f", m=M),
                                    ztB[b][:]
                                    .rearrange("p (m f) -> p m f", m=M),
                                    hi[b][:].unsqueeze(1)
                                    .to_broadcast([128, M, 128]), ALU.add)
                            tv = t[:].rearrange("p (m f) -> p m f", m=M)
                            if debug and L == 0 and b == 0:
                                for zc in range(2):
                                    zdbg = dbp.tile([128, M * 64], f32,
                                                    tag="zdbg",
                                                    name=f"zdbg{zc}")
                                    nc.vector.tensor_copy(
                                        zdbg[:],
                                        t[:, zc * M * 64:(zc + 1) * M * 64])
                                    nc.sync.dma_start(
                                        dbg["z1"].ap()
                                        [:, zc * M * 64:(zc + 1) * M * 64],
                                        zdbg[:])
                            mu = work.tile([128, M], bf16, tag="mu")
                            nc.vector.tensor_reduce(mu[:], tv, axis=AX.X,
                                                    op=ALU.add)
                            xm[b] = cvp.tile([128, M * 128], bf16,
                                             tag=f"xm{b}", name=f"xm{L}{b}")
                            xv = xm[b][:].rearrange("p (m f) -> p m f", m=M)
                            nc.vector.scalar_tensor_tensor(
                                xv,
                                mu[:].unsqueeze(2)
                                .to_broadcast([128, M, 128]),
                                -1.0 / 128.0, tv, ALU.mult, ALU.add)
                            # lt[b] doubles as the x^2 scratch before Abs
                            lt[b] = cvp.tile([128, M * 128], bf16,
                                             tag=f"lt{b}", name=f"lt{L}{b}")
                            nc.vector.tensor_tensor(lt[b][:], xm[b][:],
                                                    xm[b][:], ALU.mult)
                            vv = work.tile([128, M], bf16, tag="vv")
                            nc.vector.tensor_reduce(
                                vv[:],
                                lt[b][:].rearrange("p (m f) -> p m f", m=M),
                                axis=AX.X, op=ALU.add)
                            lv[b] = cvp.tile([128, M], f32, tag=f"lv{b}",
                                             name=f"lv{L}{b}")
                            nc.vector.tensor_copy(lv[b][:], vv[:])
                        # phase 2 (ACT batched): rsd = exp(-0.5 ln(v+eps))
                        for b in range(NB):
                            nc.scalar.activation(lv[b][:], lv[b][:], AF.Ln,
                                                 scale=1.0 / 128.0,
                                                 bias=epsc[:])
                        for b in range(NB):
                            rsd[b] = cvp.tile([128, M], bf16,
                                              tag=f"rsd{b}",
                                              name=f"rsd{L}{b}")
                            nc.scalar.activation(rsd[b][:], lv[b][:],
                                                 AF.Exp, scale=-0.5)
                        # phase 3 (DVE): normalize
                        for b in range(NB):
                            xv = xm[b][:].rearrange("p (m f) -> p m f", m=M)
                            nc.vector.tensor_tensor(
                                xv, xv,
                                rsd[b][:].unsqueeze(2)
                                .to_broadcast([128, M, 128]), ALU.mult)
                        # phase 4 (ACT batched): l = ln(1+exp(-|x|))
                        for b in range(NB):
                            nc.scalar.activation(lt[b][:], xm[b][:], AF.Abs)
                        for b in range(NB):
                            nc.scalar.activation(lt[b][:], lt[b][:], AF.Exp,
                                                 scale=-1.0)
                        for b in range(NB):
                            nc.scalar.activation(lt[b][:], lt[b][:], AF.Ln,
                                                 bias=1.0)
                        # phase 5: sig = exp(min(f,0)-l_f); sp = relu(c)+l_c
                        for b in range(NB):
                            xv = xm[b][:].rearrange("p (m f) -> p m f", m=M)
                            lv_ = lt[b][:].rearrange("p (m f) -> p m f",
                                                     m=M)
                            ug[b] = cvp.tile([128, M * FEA], bf16,
                                             tag=f"ug{b}", name=f"ug{L}{b}")
                            nc.vector.scalar_tensor_tensor(
                                ug[b][:].rearrange("p (m f) -> p m f", m=M),
                                xv[:, :, 0:FEA], 0.0,
                                lv_[:, :, 0:FEA], ALU.min, ALU.subtract)
                        for b in range(NB):
                            nc.scalar.activation(ug[b][:], ug[b][:], AF.Exp)
                        for b in range(NB):
                            xv = xm[b][:].rearrange("p (m f) -> p m f", m=M)
                            lv_ = lt[b][:].rearrange("p (m f) -> p m f",
                                                     m=M)
                            sp = work.tile([128, M * FEA], bf16, tag="sps")
                            spv = sp[:].rearrange("p (m f) -> p m f", m=M)
                            nc.vector.scalar_tensor_tensor(
                                spv, xv[:, :, FEA:128], 0.0,
                                lv_[:, :, FEA:128], ALU.max, ALU.add)
                            nc.vector.tensor_tensor(ug[b][:], ug[b][:],
                                                    sp[:], ALU.mult)
                            u_ = ug[b]
                            ns = work.tile([128, FEA], f32, tag="ns")
                            nc.vector.tensor_tensor(
                                u_[:, 0:6 * FEA], u_[:, 0:6 * FEA],
                                u_[:, 6 * FEA:12 * FEA], ALU.add)
                            nc.vector.tensor_tensor(
                                u_[:, 0:3 * FEA], u_[:, 0:3 * FEA],
                                u_[:, 3 * FEA:6 * FEA], ALU.add)
                            nc.vector.tensor_tensor(
                                u_[:, 0:FEA], u_[:, 0:FEA],
                                u_[:, FEA:2 * FEA], ALU.add)
                            nc.vector.tensor_tensor(
                                ns[:], u_[:, 0:FEA],
                                u_[:, 2 * FEA:3 * FEA], ALU.add)
                            att[b] = cvp.tile([128, FEA], f32,
                                              tag=f"at{b}",
                                              name=f"at{L}{b}")
                            nc.vector.tensor_tensor(att[b][:], aprev[b][:],
                                                    ns[:], ALU.add)
                        # phase 6 (ACT batched): atom softplus
                        for b in range(NB):
                            spa[b] = cvp.tile([128, FEA], f32,
                                              tag=f"spa{b}",
                                              name=f"spa{L}{b}")
                            nc.scalar.activation(spa[b][:], att[b][:],
                                                 AF.Abs)
                        for b in range(NB):
                            nc.scalar.activation(spa[b][:], spa[b][:],
                                                 AF.Exp, scale=-1.0)
                        for b in range(NB):
                            nc.scalar.activation(spa[b][:], spa[b][:],
                                                 AF.Ln, bias=1.0)
                        for b in range(NB):
                            nc.vector.scalar_tensor_tensor(
                                anext[b][:], att[b][:], 0.0, spa[b][:],
                                ALU.max, ALU.add)
                        if L == 0:
                            # hi2 from atom1 (ones row -> exact bias fold)
                            a1x = epool.tile([FEA + 1, NL], f32, tag="a1x")
                            nc.vector.memset(a1x[FEA:FEA + 1, :], 1.0)
                            for b in range(NB):
                                sl = slice(b * 128, (b + 1) * 128)
                                tp = psCg.tile([FEA, 128], f32, tag="tpC")
                                nc.tensor.transpose(tp[:], atom1[b][:],
                                                    ident[:])
                                nc.scalar.activation(a1x[0:FEA, sl], tp[:],
                                                     AF.Copy)
                                hp = psCg.tile([128, 2 * FEA], f32,
                                               tag="hpC")
                                nc.tensor.matmul(hp[:], a1x[:, sl], wib2[:],
                                                 start=True, stop=True)
                                nc.vector.tensor_copy(hi2[b][:], hp[:])
                            if debug:
                                for b in range(NB):
                                    nc.sync.dma_start(
                                        dbg["atom1"].ap()
                                        [b * 128:(b + 1) * 128, :],
                                        atom1[b][:])

                for b in range(NB):
                    nc.sync.dma_start(d_out.ap()[b * 128:(b + 1) * 128, :],
                                      atom2[b][:])

    _body()
    nc.compile()
    return nc


def _prep_inputs(inputs):
    """Host-side layout prep. Returns (in_maps, host_ctx)."""
    import ml_dtypes
    bf = ml_dtypes.bfloat16
    f32 = np.float32
    lat = np.asarray(inputs["lat_pred"], f32)
    fr = np.ascontiguousarray(np.asarray(inputs["fracs_pred"], f32))
    sl = np.ascontiguousarray(np.asarray(inputs["species_logits"], f32))
    occ = np.asarray(inputs["occ_logits"], f32)
    emb_w = np.asarray(inputs["emb_w"], f32)
    emb_b = np.asarray(inputs["emb_b"], f32)
    w1 = np.asarray(inputs["w1"], f32); b1 = np.asarray(inputs["b1"], f32)
    w2 = np.asarray(inputs["w2"], f32); b2 = np.asarray(inputs["b2"], f32)

    G = (lat.astype(np.float64) @ lat.T.astype(np.float64))
    wroot = np.sqrt(np.diag(G)).astype(f32)

    frecs = np.zeros((N, 64), f32)
    frecs[:, 0:3] = fr

    gneg = (-np.array([G[0, 0], G[1, 1], G[2, 2],
                       2 * G[0, 1], 2 * G[0, 2], 2 * G[1, 2]])).astype(f32)

    splogT = np.zeros((128, N), f32)
    splogT[0:SPECIES, :] = sl.T

    embwx = np.concatenate([emb_w, np.ones((SPECIES, 1), f32)], 1)
    # hi1 path: psum = af_un@wi1 + rs*(b1 + emb_b@wi1); * (1/rs) gives
    # (af_un/rs + emb_b)@wi1 + b1 = af@wi1 + b1 exactly.
    wib1 = np.ascontiguousarray(np.concatenate(
        [w1[0:FEA, :], (b1 + emb_b @ w1[0:FEA, :])[None, :]], 0))
    wib2 = np.ascontiguousarray(
        np.concatenate([w2[0:FEA, :], b2[None, :]], 0))
    wjx = np.ascontiguousarray(
        np.concatenate([w1[FEA:2 * FEA, :], w2[FEA:2 * FEA, :]], 1)).astype(bf)
    wnx = np.ascontiguousarray(
        np.concatenate([w1[2 * FEA:, :], w2[2 * FEA:, :]], 1)).astype(bf)

    shared = dict(
        splogT=splogT,
        fracsT=np.ascontiguousarray(fr.T),
        frecs=frecs,
        embwx=np.ascontiguousarray(embwx),
        embbrow=np.ascontiguousarray(np.broadcast_to(emb_b, (128, FEA))),
        wib1=wib1, wib2=wib2, wjx=wjx, wnx=wnx,
        gcol=np.ascontiguousarray(np.broadcast_to(gneg, (128, 6))),
        wroot=wroot.reshape(3, 1),
        noff=(-OFFSET).reshape(KG, 1),
        blockoff=np.ascontiguousarray(np.broadcast_to(
            np.repeat(np.arange(NBLK, dtype=np.uint32) * BLK, 8),
            (128, CAND))).astype(np.uint32),
        identb=np.eye(128, dtype=f32).astype(bf),
        ident=np.eye(128, dtype=f32),
    )
    in_maps = []
    for c in range(NCORES):
        rows = slice(c * NL, (c + 1) * NL)
        selfid = (c * NL + np.arange(128, dtype=f32)[:, None]
                  + 128 * np.arange(NB, dtype=f32)[None, :]).astype(f32)
        sploclT = np.zeros((128, NL), f32)
        sploclT[0:SPECIES, :] = sl[rows].T
        m = dict(shared)
        m.update(sploclT=sploclT, fl=np.ascontiguousarray(fr[rows]),
                 flT=np.ascontiguousarray(fr[rows].T),
                 selfid=np.ascontiguousarray(selfid))
        in_maps.append(m)
    host = dict(occ=occ, fc_w=np.asarray(inputs["fc_w"], f32),
                fc_b=np.asarray(inputs["fc_b"], f32))
    return in_maps, host


def _host_finish(results, host):
    a2 = np.concatenate([np.asarray(r["atom2"]) for r in results], 0)
    occp = 1.0 / (1.0 + np.exp(-host["occ"].astype(np.float64)))
    graph = (a2.astype(np.float64) * occp[:, None]).sum(0) / (occp.sum()
                                                              + 1e-6)
    out = graph @ host["fc_w"].astype(np.float64) + host["fc_b"]
    return out.astype(np.float32)


def kernel(**inputs) -> np.ndarray:
    from concourse import bass_utils

    in_maps, host = _prep_inputs(inputs)
    key = "prog"
    if key not in _cache:
        _cache[key] = _build_program(debug=False)
    nc = _cache[key]
    res = bass_utils.run_bass_kernel_spmd(nc, in_maps,
                                          core_ids=list(range(NCORES)))
    return _host_finish(res.results, host)


# revision 42
# speedup vs baseline: 1.0830x; 1.0636x over previous
"""Trainium2 Bass kernel for nn_DifferentiableCGCNN (N=4096 atoms, 8 NeuronCores).

v2 strategy (SPMD, one identical program per core, no collectives):
  - neighbor SELECTION by surrogate rank only (validated offline: end-to-end
    output shift 5e-4 << 2e-2 tol): torus surrogate dots as bf16 PE matmuls,
    top-8 per 512-block (max/max_index), then top-12 of the 64 candidates
    by value; global indices via compare-select masks.
  - exact distances only for the selected 12 (1.5MB gather of padded fracs
    rows), d12 = sqrt(e^T G e) on DVE/ACT.
  - embedding: host-transposed species logits -> Exp -> row matmuls with
    [embw|ones] so the softmax sum rides along as column 64; normalization
    folded into the PSUM evacuation (x * 1/sum + bias). bf16 atom-feature
    table [N,128] written to DRAM for the neighbor gather. Local shard is
    handled feature-major from a per-core sploclT input; the sum row rides
    as partition 64 so the bias folds exactly ((af_un@W + rs*b) * 1/rs).
  - neighbor features: a premultiplied hjW = af @ [wj1|wj2] table (bf16,
    512B rows) is gathered atom-major with plain dma_gather (1024-idx
    chunks; transpose-mode gather and >1024-descriptor gathers CRASH on
    real HW) and added into PSUM via one wide identity matmul; gaussians
    are built feature-major and contracted with [wn1|wn2] per (b,m).
  - LN classic (reduce + fused mean-subtract); sigmoid and softplus share
    one Abs/Exp/Ln pass (sig = exp(min(f,0)-l_f), sp = relu(c)+l_c) so all
    stage-C ACT ops use exp/ln-capable tables; ops are batched per phase
    across the four row tiles because the act-table chooser is greedy and
    an Exp<->Ln ping-pong costs 1.28us per switch. Elementwise in bf16
    for the DVE 2x mode; stage E's Exp carries a zero bias data-dependent
    on the Sin outputs to keep the sin/exp tables from thrashing.
  - final occupancy pooling + fc on host from per-core [512,64] outputs.
"""
import os
import sys

import numpy as np

for _p in ("/opt/trn_rl_repo",):
    if os.path.isdir(_p) and _p not in sys.path:
        sys.path.insert(0, _p)

N = 4096
NCORES = 8
NL = N // NCORES          # 512 atoms per core
NB = NL // 128            # 4 row tiles per core
SPECIES = 100
FEA = 64
KG = 64                   # gaussian filters
M = 12                    # neighbors
BLK = 512                 # surrogate block width
NBLK = N // BLK           # 8 blocks
CAND = NBLK * 8           # 64 candidates per row
LN_EPS = 1e-5
BIG = 1e30

OFFSET = np.linspace(0.0, 8.0, KG).astype(np.float32)
COEFF = float(-0.5 / (8.0 / (KG - 1)) ** 2)

_cache = {}


def _build_program(debug: bool = False):
    from contextlib import ExitStack

    import concourse.bacc as bacc
    import concourse.mybir as mybir
    from concourse.tile import TileContext

    dt = mybir.dt
    AF = mybir.ActivationFunctionType
    ALU = mybir.AluOpType
    AX = mybir.AxisListType
    f32 = dt.float32
    bf16 = dt.bfloat16
    f32r = dt.float32r

    nc = bacc.Bacc("TRN2", target_bir_lowering=False, debug=False,
                   enable_asserts=False)

    # ---- dram inputs ----
    d_splogT = nc.dram_tensor("splogT", [128, N], f32, kind="ExternalInput")
    d_sploclT = nc.dram_tensor("sploclT", [128, NL], f32,
                               kind="ExternalInput")
    d_uT = nc.dram_tensor("uTh", [36, N], bf16, kind="ExternalInput")
    d_uloc = nc.dram_tensor("uloch", [36, NL], bf16, kind="ExternalInput")
    d_fl = nc.dram_tensor("fl", [NL, 3], f32, kind="ExternalInput")
    d_frecs = nc.dram_tensor("frecs", [N, 64], f32, kind="ExternalInput")
    d_embwx = nc.dram_tensor("embwx", [SPECIES, FEA + 1], f32,
                             kind="ExternalInput")
    d_embbrow = nc.dram_tensor("embbrow", [128, FEA], f32,
                               kind="ExternalInput")
    d_wib1 = nc.dram_tensor("wib1", [FEA + 1, 2 * FEA], f32,
                            kind="ExternalInput")
    d_wib2 = nc.dram_tensor("wib2", [FEA + 1, 2 * FEA], f32,
                            kind="ExternalInput")
    d_wjx = nc.dram_tensor("wjx", [FEA, 4 * FEA], bf16, kind="ExternalInput")
    d_wnx = nc.dram_tensor("wnx", [FEA, 4 * FEA], bf16, kind="ExternalInput")
    d_gcol = nc.dram_tensor("gcol", [128, 6], f32, kind="ExternalInput")
    d_wroot = nc.dram_tensor("wroot", [3, 1], f32, kind="ExternalInput")
    d_noff = nc.dram_tensor("noff", [KG, 1], f32, kind="ExternalInput")
    d_selfid = nc.dram_tensor("selfid", [128, NB], f32, kind="ExternalInput")
    d_blockoff = nc.dram_tensor("blockoff", [128, CAND], dt.uint32,
                                kind="ExternalInput")
    d_identb = nc.dram_tensor("identb", [128, 128], bf16, kind="ExternalInput")
    d_ident = nc.dram_tensor("ident", [128, 128], f32, kind="ExternalInput")

    d_out = nc.dram_tensor("atom2", [NL, FEA], f32, kind="ExternalOutput")

    dbg = {}
    if debug:
        def dbg_t(name, shape, dtyp=f32):
            dbg[name] = nc.dram_tensor("dbg_" + name, shape, dtyp,
                                       kind="ExternalOutput")
        dbg_t("mxall", [128, NB * CAND])
        dbg_t("nidx", [128, NB * M], dt.uint32)
        dbg_t("d12", [128, NB * M])
        dbg_t("af0", [128, FEA])
        dbg_t("hi1", [128, 2 * FEA])
        dbg_t("hjg", [128, 512])
        dbg_t("gauss", [KG, 512])
        dbg_t("z1", [128, M * 128])
        dbg_t("atom1", [NL, FEA])

    def _body():  # noqa: PLR0915
        with TileContext(nc) as tc, \
             nc.allow_low_precision(reason="bf16 staging is intentional"):
            with ExitStack() as ctx:
                const = ctx.enter_context(tc.tile_pool(name="const", bufs=1))
                persist = ctx.enter_context(tc.tile_pool(name="persist",
                                                         bufs=1))
                work = ctx.enter_context(tc.tile_pool(name="work", bufs=4))
                spool = ctx.enter_context(tc.tile_pool(name="spool", bufs=1))
                epool = ctx.enter_context(tc.tile_pool(name="epool", bufs=1))
                ework = ctx.enter_context(tc.tile_pool(name="ework", bufs=4))
                gpool = ctx.enter_context(tc.tile_pool(name="gpool", bufs=1))
                cvp = ctx.enter_context(tc.tile_pool(name="cvp", bufs=1))
                dbp = ctx.enter_context(tc.tile_pool(name="dbp", bufs=1))
                drp = ctx.enter_context(tc.tile_pool(name="dram", bufs=1,
                                                     space="DRAM"))

                def cload(dram, shape, dtyp=f32):
                    t = const.tile(shape, dtyp, tag=dram.name + "_c")
                    nc.sync.dma_start(t[:], dram.ap())
                    return t

                embwx = cload(d_embwx, [SPECIES, FEA + 1])
                embbrow = cload(d_embbrow, [128, FEA])
                wib1 = cload(d_wib1, [FEA + 1, 2 * FEA])
                wib2 = cload(d_wib2, [FEA + 1, 2 * FEA])
                wjx = cload(d_wjx, [FEA, 4 * FEA], bf16)
                wnx = cload(d_wnx, [FEA, 4 * FEA], bf16)
                gcol = cload(d_gcol, [128, 6])
                wroot = cload(d_wroot, [3, 1])
                gsq = const.tile([3, 1], f32, tag="gsq")
                nc.vector.tensor_tensor(gsq[:], wroot[:], wroot[:], ALU.mult)
                noff = cload(d_noff, [KG, 1])
                selfid = cload(d_selfid, [128, NB])
                blockoff = cload(d_blockoff, [128, CAND], dt.uint32)
                identb = cload(d_identb, [128, 128], bf16)
                ident = cload(d_ident, [128, 128])
                mpi = const.tile([3, 1], f32, tag="mpi")
                nc.vector.memset(mpi[:], -np.pi)
                hpi = const.tile([3, 1], f32, tag="hpi")
                nc.vector.memset(hpi[:], np.pi / 2)
                epsc = const.tile([128, 1], f32, tag="epsc")
                nc.vector.memset(epsc[:], LN_EPS)

                # dram scratch
                hjw = drp.tile([N, 4 * FEA], bf16, tag="hjw")
                dflat2 = drp.tile([NB * M, 128], f32, tag="dflat2")
                hbmI = drp.tile([16, NB * M * 8], dt.int16, tag="hbmI")

                # persistent cross-stage tiles
                nidx16 = persist.tile([128, NB * M], dt.int16, tag="nidx16")
                d12 = persist.tile([128, NB * M], f32, tag="d12")
                hjwg = persist.tile([128, NB * M * 4 * FEA], bf16,
                                    tag="hjwg")
                gss = persist.tile([KG, NB * M * 128], bf16, tag="gss")
                flb = [persist.tile([128, 3], f32, tag=f"flb_{b}",
                                    name=f"flb_{b}") for b in range(NB)]
                rrloc = [persist.tile([128, 1], f32, tag=f"rr_{b}",
                                      name=f"rr_{b}") for b in range(NB)]
                atom0 = [persist.tile([128, FEA], f32, tag=f"a0_{b}",
                                      name=f"a0_{b}") for b in range(NB)]
                atom1 = [persist.tile([128, FEA], f32, tag=f"a1_{b}",
                                      name=f"a1_{b}") for b in range(NB)]
                atom2 = [persist.tile([128, FEA], f32, tag=f"a2_{b}",
                                      name=f"a2_{b}") for b in range(NB)]
                hi1 = [persist.tile([128, 2 * FEA], bf16, tag=f"hi1_{b}",
                                    name=f"hi1_{b}") for b in range(NB)]
                hi2 = [persist.tile([128, 2 * FEA], bf16, tag=f"hi2_{b}",
                                    name=f"hi2_{b}") for b in range(NB)]

                # ======== stages S (select) + E (embed) + G (gather) ======
                uT = spool.tile([36, N], bf16, tag="uT")
                uloc = spool.tile([36, NL], bf16, tag="uloc")
                fT = spool.tile([3, N], f32, tag="fT")
                fTl = spool.tile([3, NL], f32, tag="fTl")
                mxall = spool.tile([128, NB * CAND], f32, tag="mxall")
                idxgF = spool.tile([128, NB * CAND], f32, tag="idxgF")
                idxg = spool.tile([128, NB * CAND], dt.uint32, tag="idxg")
                vals = spool.tile([128, NB * 16], f32, tag="vals")
                nidxF = spool.tile([128, NB * M], f32, tag="nidxF")
                msk = spool.tile([128, NB * M * CAND], f32, tag="msk")

                expT = epool.tile([128, N], f32, tag="expT")
                explT = epool.tile([128, NL], f32, tag="explT")
                afTx = epool.tile([FEA + 1, NL], f32, tag="afTx")

                with tc.tile_pool(name="psS", bufs=2, space="PSUM") as psS, \
                     tc.tile_pool(name="psA", bufs=1, space="PSUM") as psA, \
                     tc.tile_pool(name="psL", bufs=1, space="PSUM") as psL, \
                     tc.tile_pool(name="psT", bufs=1, space="PSUM") as psT:

                    # ---- stage S ----
                    nc.sync.dma_start(uT[:], d_uT.ap())
                    nc.sync.dma_start(uloc[:], d_uloc.ap())
                    for b in range(NB):
                        nc.sync.dma_start(flb[b][:],
                                          d_fl.ap()[b * 128:(b + 1) * 128, :])

                    idxg_v = idxg[:].rearrange("p (b c) -> p b c", b=NB)
                    mx_v = mxall[:].rearrange("p (b c) -> p b c", b=NB)

                    for b in range(NB):
                        for j in range(8):
                            dps = psS.tile([128, 512], f32, tag="dots")
                            nc.tensor.matmul(
                                dps[:], uloc[:, b * 128:(b + 1) * 128],
                                uT[:, j * 512:(j + 1) * 512],
                                start=True, stop=True)
                            nc.vector.max(
                                out=mx_v[:, b, j * 8:(j + 1) * 8], in_=dps[:])
                            nc.vector.max_index(
                                idxg_v[:, b, j * 8:(j + 1) * 8],
                                mx_v[:, b, j * 8:(j + 1) * 8], dps[:])

                    nc.vector.tensor_tensor(
                        idxg_v, idxg_v,
                        blockoff[:].unsqueeze(1).to_broadcast([128, NB, CAND]),
                        ALU.add)
                    nc.vector.tensor_copy(idxgF[:], idxg[:])

                    # self-exclusion on candidate values
                    smsk = work.tile([128, NB * CAND], f32, tag="smsk")
                    nc.vector.tensor_tensor(
                        smsk[:].rearrange("p (b c) -> p b c", b=NB),
                        idxgF[:].rearrange("p (b c) -> p b c", b=NB),
                        selfid[:].unsqueeze(2).to_broadcast([128, NB, CAND]),
                        ALU.is_equal)
                    nc.vector.scalar_tensor_tensor(mxall[:], smsk[:], -BIG,
                                                   mxall[:], ALU.mult,
                                                   ALU.add)
                    if debug:
                        nc.sync.dma_start(dbg["mxall"].ap(), mxall[:])

                    # top-12 of 64 per tile by value
                    for b in range(NB):
                        seg = mxall[:, b * CAND:(b + 1) * CAND]
                        nc.vector.max(out=vals[:, b * 16:b * 16 + 8], in_=seg)
                        mr = work.tile([128, CAND], f32, tag="mr")
                        nc.vector.match_replace(
                            out=mr[:],
                            in_to_replace=vals[:, b * 16:b * 16 + 8],
                            in_values=seg, imm_value=-BIG)
                        nc.vector.max(out=vals[:, b * 16 + 8:b * 16 + 16],
                                      in_=mr[:])

                    # global indices of the selected 12 via compare-select
                    vals_v = vals[:].rearrange("p (b v) -> p b v", b=NB)
                    msk_v = msk[:].rearrange("p (b m c) -> p b m c", b=NB,
                                             m=M)
                    nc.vector.tensor_tensor(
                        msk_v,
                        mxall[:].rearrange("p (b c) -> p b c", b=NB)
                        .unsqueeze(2).to_broadcast([128, NB, M, CAND]),
                        vals_v[:, :, 0:M].unsqueeze(3)
                        .to_broadcast([128, NB, M, CAND]),
                        ALU.is_equal)
                    nc.vector.tensor_tensor(
                        msk_v, msk_v,
                        idxgF[:].rearrange("p (b c) -> p b c", b=NB)
                        .unsqueeze(2).to_broadcast([128, NB, M, CAND]),
                        ALU.mult)
                    nc.vector.tensor_reduce(
                        nidxF[:].rearrange("p (b m) -> p b m", b=NB), msk_v,
                        axis=AX.X, op=ALU.max)
                    nc.vector.tensor_copy(nidx16[:], nidxF[:])
                    if debug:
                        nidxU = dbp.tile([128, NB * M], dt.uint32,
                                          tag="nidxU")
                        nc.vector.tensor_copy(nidxU[:], nidxF[:])
                        nc.sync.dma_start(dbg["nidx"].ap(), nidxU[:])

                    # ---- stage E (independent of S; overlaps it) ----
                    for h in range(8):
                        sl = slice(h * 512, (h + 1) * 512)
                        nc.sync.dma_start(expT[:, sl], d_splogT.ap()[:, sl])
                        nc.scalar.activation(expT[0:SPECIES, sl],
                                             expT[0:SPECIES, sl], AF.Exp)
                    nc.sync.dma_start(explT[:], d_sploclT.ap())
                    nc.scalar.activation(explT[0:SPECIES, :],
                                         explT[0:SPECIES, :], AF.Exp)

                    # full-graph neighbor table: hjW rows =
                    # af_row @ [wj1|wj2]  (af normalized + emb-biased)
                    for c in range(N // 128):
                        ps = psA.tile([128, FEA + 1], f32, tag="psA")
                        nc.tensor.matmul(
                            ps[:], expT[0:SPECIES, c * 128:(c + 1) * 128],
                            embwx[:], start=True, stop=True)
                        rr = ework.tile([128, 1], f32, tag="rr")
                        nc.vector.reciprocal(rr[:], ps[:, FEA:FEA + 1])
                        ab = ework.tile([128, FEA], bf16, tag="ab")
                        nc.vector.scalar_tensor_tensor(
                            ab[:], ps[:, 0:FEA], rr[:], embbrow[:],
                            ALU.mult, ALU.add)
                        tp2 = psA.tile([FEA, 128], bf16, tag="tp2")
                        nc.tensor.transpose(tp2[:], ab[:], identb[:])
                        abT = ework.tile([FEA, 128], bf16, tag="abT")
                        nc.vector.tensor_copy(abT[:], tp2[:])
                        psW = psA.tile([128, 4 * FEA], f32, tag="psW")
                        nc.tensor.matmul(psW[:], abT[:], wjx[:],
                                         start=True, stop=True)
                        hw_ = ework.tile([128, 4 * FEA], bf16, tag="hw_")
                        nc.vector.tensor_copy(hw_[:], psW[:])
                        nc.sync.dma_start(hjw[c * 128:(c + 1) * 128, :],
                                          hw_[:])

                    # local feature-major [af_unnorm ; sums] via f32r matmuls
                    psl = psL.tile([FEA + 1, NL], f32, tag="psl")
                    nc.tensor.matmul(psl[0:FEA, :],
                                     embwx[:, 0:FEA],
                                     explT[0:SPECIES, :],
                                     start=True, stop=True)
                    nc.tensor.matmul(psl[FEA:FEA + 1, :],
                                     embwx[:, FEA:FEA + 1],
                                     explT[0:SPECIES, :],
                                     start=True, stop=True)
                    nc.scalar.activation(afTx[:], psl[:], AF.Copy)

                    # per-tile: rows (atom0, rr) via PE transpose; hi1 matmul
                    for b in range(NB):
                        sl = slice(b * 128, (b + 1) * 128)
                        tp = psT.tile([128, FEA + 1], f32, tag="tpE")
                        nc.tensor.transpose(tp[:], afTx[:, sl],
                                            ident[0:FEA + 1, 0:FEA + 1])
                        nc.vector.reciprocal(rrloc[b][:], tp[:, FEA:FEA + 1])
                        nc.vector.scalar_tensor_tensor(
                            atom0[b][:], tp[:, 0:FEA], rrloc[b][:],
                            embbrow[:], ALU.mult, ALU.add)
                        hp = psT.tile([128, 2 * FEA], f32, tag="hpE")
                        nc.tensor.matmul(hp[:], afTx[:, sl], wib1[:],
                                         start=True, stop=True)
                        nc.vector.tensor_scalar_mul(hi1[b][:], hp[:],
                                                    rrloc[b][:])
                    if debug:
                        nc.sync.dma_start(dbg["af0"].ap(), atom0[0][:])
                        hj1f = dbp.tile([128, 2 * FEA], f32, tag="hj1f")
                        nc.vector.tensor_copy(hj1f[:], hi1[0][:])
                        nc.sync.dma_start(dbg["hi1"].ap(), hj1f[:])

                    # ---- stage G: gathers + d12 + gauss ----
                    hv = hbmI[:].rearrange("s (c e) -> s c e", e=8)
                    for w in range(8):
                        nc.sync.dma_start(hv[:, :, w],
                                          nidx16[16 * w:16 * (w + 1), :])
                    idxsG = gpool.tile([128, NB * M * 8], dt.int16,
                                       tag="idxsG")
                    for r in range(8):
                        nc.sync.dma_start(idxsG[16 * r:16 * (r + 1), :],
                                          hbmI[:])

                    # fracs gather: [128, 48, 64] f32 (256B rows)
                    crec = gpool.tile([128, NB * M * 64], f32, tag="crec")
                    crec_ch = crec[:].rearrange("p (c e) -> p c e", e=64)
                    for k in range(6):
                        nc.gpsimd.dma_gather(
                            crec_ch[:, k * 8:(k + 1) * 8, :], d_frecs.ap(),
                            idxsG[:, k * 64:(k + 1) * 64], 1024, 1024, 64)

                    # neighbor z-contribution gather (512B bf16 rows,
                    # atom-major — added to PSUM via one identity matmul)
                    hjwg_ch = hjwg[:].rearrange("p (c e) -> p c e",
                                                e=4 * FEA)
                    for k in range(6):
                        nc.gpsimd.dma_gather(
                            hjwg_ch[:, k * 8:(k + 1) * 8, :], hjw[:],
                            idxsG[:, k * 64:(k + 1) * 64],
                            1024, 1024, 4 * FEA)
                    if debug:
                        hjdbg = dbp.tile([128, 512], f32, tag="hjdbg")
                        nc.vector.tensor_copy(hjdbg[:], hjwg[:, 0:512])
                        nc.sync.dma_start(dbg["hjg"].ap(), hjdbg[:])

                    # ---- d12 (exact metric for the selected 12) ----
                    da = [gpool.tile([128, NB * M], f32, tag=f"da{a}",
                                     name=f"da{a}") for a in range(3)]
                    for b in range(NB):
                        for a in range(3):
                            nc.vector.tensor_scalar(
                                da[a][:, b * M:(b + 1) * M],
                                crec_ch[:, b * M:(b + 1) * M, a],
                                flb[b][:, a:a + 1], None, op0=ALU.subtract)
                    W = NB * M
                    for a in range(3):
                        u1 = work.tile([128, W], f32, tag="u1",
                                       name=f"u1{a}")
                        nc.vector.scalar_tensor_tensor(u1[:], da[a][:], 0.5,
                                                       da[a][:], ALU.is_gt,
                                                       ALU.subtract)
                        nc.vector.scalar_tensor_tensor(da[a][:], da[a][:],
                                                       -0.5, u1[:],
                                                       ALU.is_lt,
                                                       ALU.subtract)
                    terms = [(0, 0, 0), (1, 1, 1), (2, 2, 2),
                             (0, 1, 3), (0, 2, 4), (1, 2, 5)]
                    acc = gpool.tile([128, W], f32, tag="acc")
                    accb = gpool.tile([128, W], f32, tag="accb")
                    cur, nxt = acc, accb
                    for i, (ia, ib, gi) in enumerate(terms):
                        pr = work.tile([128, W], f32, tag="pr",
                                       name=f"pr{i}")
                        nc.vector.tensor_tensor(pr[:], da[ia][:], da[ib][:],
                                                ALU.mult)
                        if i == 0:
                            nc.vector.tensor_scalar_mul(cur[:], pr[:],
                                                        gcol[:, 0:1])
                        else:
                            nc.vector.scalar_tensor_tensor(
                                nxt[:], pr[:], gcol[:, gi:gi + 1], cur[:],
                                ALU.mult, ALU.add)
                            cur, nxt = nxt, cur
                    # gcol holds -G entries (cur = -d^2); d12 = sqrt(-cur)
                    nc.vector.tensor_scalar_min(cur[:], cur[:], -1e-12)
                    nc.scalar.activation(cur[:], cur[:], AF.Ln, scale=-1.0)
                    nc.scalar.activation(d12[:], cur[:], AF.Exp, scale=0.5)
                    if debug:
                        nc.sync.dma_start(dbg["d12"].ap(), d12[:])

                    # d12 -> DRAM slot-major -> broadcast -> gaussians
                    nc.sync.dma_start(dflat2[:].transpose([1, 0]), d12[:])
                    dfb = (dflat2[:].rearrange("c p -> (c p)").unsqueeze(0)
                           .to_broadcast([KG, NB * M * 128]))
                    gin = gpool.tile([KG, NB * M * 128], f32, tag="gin")
                    nc.sync.dma_start(gin[:], dfb)
                    nc.scalar.activation(gin[:], gin[:], AF.Square,
                                         bias=noff[:])
                    nc.scalar.activation(gss[:], gin[:], AF.Exp,
                                         scale=COEFF)
                    if debug:
                        gdbg = dbp.tile([KG, 512], f32, tag="gdbg")
                        nc.vector.tensor_copy(gdbg[:], gss[:, 0:512])
                        nc.sync.dma_start(dbg["gauss"].ap(), gdbg[:])

                # ================= stage C: conv layers ===================
                def softplus_ln(out_ap, in_ap, pool, shape, tag, dtyp):
                    """out = relu(x) + ln(1 + exp(-|x|)); ACT: Abs,Exp,Ln."""
                    t = pool.tile(shape, dtyp, tag="sptmp",
                                  name=tag + "_t")
                    nc.scalar.activation(t[:], in_ap, AF.Abs)
                    nc.scalar.activation(t[:], t[:], AF.Exp, scale=-1.0)
                    nc.scalar.activation(t[:], t[:], AF.Ln, bias=1.0)
                    nc.vector.scalar_tensor_tensor(out_ap, in_ap, 0.0, t[:],
                                                   ALU.max, ALU.add)

                # per b: psum zz[p,(m,256)] = sum_m gauss_m @ [wn1|wn2]
                #        + ident @ hjWg_b (both layers' neighbor term)
                # evac once to bf16; per layer: +hi, LN, sigmoid*softplus
                # (all ACT ops live in the exp/ln table set).
                gss_v = gss[:].rearrange("k (b m a) -> k b m a", b=NB, m=M)

                with tc.tile_pool(name="psCz", bufs=1, space="PSUM") as psCz, \
                     tc.tile_pool(name="psCg", bufs=1, space="PSUM") as psCg:
                    zt = [None] * NB
                    for b in range(NB):
                        zz = psCz.tile([128, M * 4 * FEA], f32, tag="zz")
                        for m in range(M):
                            nc.tensor.matmul(
                                zz[:, m * 256:(m + 1) * 256],
                                gss_v[:, b, m, :], wnx[:],
                                start=True, stop=False)
                            nc.tensor.matmul(
                                zz[:, m * 256:(m + 1) * 256], identb[:],
                                hjwg[:, (b * M + m) * 256:
                                     (b * M + m + 1) * 256],
                                start=False, stop=True)
                        zt[b] = cvp.tile([128, M * 4 * FEA], bf16,
                                         tag=f"zt{b}", name=f"zt{b}")
                        nc.scalar.activation(zt[b][:], zz[:], AF.Copy)

                    for L in range(2):
                        hi = hi1 if L == 0 else hi2
                        aprev = atom0 if L == 0 else atom1
                        anext = atom1 if L == 0 else atom2
                        xm = [None] * NB
                        lt = [None] * NB
                        ug = [None] * NB
                        lv = [None] * NB
                        rsd = [None] * NB
                        att = [None] * NB
                        spa = [None] * NB
                        # phase 1 (DVE only): t=z+hi, mu, xm, sq->lt, vv->lv
                        for b in range(NB):
                            if L == 0:
                                t = tL1[b]
                            else:
                                t = cvp.tile([128, M * 128], bf16,
                                             tag="tcs", name=f"tc{L}{b}")
                                nc.vector.tensor_tensor(
                                    t[:].rearrange("p (m f) -> p m f", m=M),
                                    ztB[b][:]
                                    .rearrange("p (m f) -> p m f", m=M),
                                    hi[b][:].unsqueeze(1)
                                    .to_broadcast([128, M, 128]), ALU.add)
                            tv = t[:].rearrange("p (m f) -> p m f", m=M)
                            if debug and L == 0 and b == 0:
                                for zc in range(2):
                                    zdbg = dbp.tile([128, M * 64], f32,
                                                    tag="zdbg",
                                                    name=f"zdbg{zc}")
                                    nc.vector.tensor_copy(
                                        zdbg[:],
                                        t[:, zc * M * 64:(zc + 1) * M * 64])
                                    nc.sync.dma_start(
                                        dbg["z1"].ap()
                                        [:, zc * M * 64:(zc + 1) * M * 64],
                                        zdbg[:])
                            mu = work.tile([128, M], bf16, tag="mu")
                            nc.vector.tensor_reduce(mu[:], tv, axis=AX.X,
                                                    op=ALU.add)
                            xm[b] = cvp.tile([128, M * 128], bf16,
                                             tag=f"xm{b}", name=f"xm{L}{b}")
                            xv = xm[b][:].rearrange("p (m f) -> p m f", m=M)
                            nc.vector.scalar_tensor_tensor(
                                xv,
                                mu[:].unsqueeze(2)
                                .to_broadcast([128, M, 128]),
                                -1.0 / 128.0, tv, ALU.mult, ALU.add)
                            # lt[b] doubles as the x^2 scratch before Abs
                            lt[b] = cvp.tile([128, M * 128], bf16,
                                             tag=f"lt{b}", name=f"lt{L}{b}")
                            nc.vector.tensor_tensor(lt[b][:], xm[b][:],
                                                    xm[b][:], ALU.mult)
                            vv = work.tile([128, M], bf16, tag="vv")
                            nc.vector.tensor_reduce(
                                vv[:],
                                lt[b][:].rearrange("p (m f) -> p m f", m=M),
                                axis=AX.X, op=ALU.add)
                            lv[b] = cvp.tile([128, M], f32, tag=f"lv{b}",
                                             name=f"lv{L}{b}")
                            nc.vector.tensor_copy(lv[b][:], vv[:])
                        # phase 2 (ACT batched): rsd = exp(-0.5 ln(v+eps))
                        for b in range(NB):
                            nc.scalar.activation(lv[b][:], lv[b][:], AF.Ln,
                                                 scale=1.0 / 128.0,
                                                 bias=epsc[:])
                        for b in range(NB):
                            rsd[b] = cvp.tile([128, M], bf16,
                                              tag=f"rsd{b}",
                                              name=f"rsd{L}{b}")
                            nc.scalar.activation(rsd[b][:], lv[b][:],
                                                 AF.Exp, scale=-0.5)
                        # phase 3 (DVE): normalize
                        for b in range(NB):
                            xv = xm[b][:].rearrange("p (m f) -> p m f", m=M)
                            nc.vector.tensor_tensor(
                                xv, xv,
                                rsd[b][:].unsqueeze(2)
                                .to_broadcast([128, M, 128]), ALU.mult)
                        # phase 4 (ACT batched): l = ln(1+exp(-|x|))
                        for b in range(NB):
                            nc.scalar.activation(lt[b][:], xm[b][:], AF.Abs)
                        for b in range(NB):
                            nc.scalar.activation(lt[b][:], lt[b][:], AF.Exp,
                                                 scale=-1.0)
                        for b in range(NB):
                            nc.scalar.activation(lt[b][:], lt[b][:], AF.Ln,
                                                 bias=1.0)
                        # phase 5: sig = exp(min(f,0)-l_f); sp = relu(c)+l_c
                        for b in range(NB):
                            xv = xm[b][:].rearrange("p (m f) -> p m f", m=M)
                            lv_ = lt[b][:].rearrange("p (m f) -> p m f",
                                                     m=M)
                            ug[b] = cvp.tile([128, M * FEA], bf16,
                                             tag=f"ug{b}", name=f"ug{L}{b}")
                            nc.vector.scalar_tensor_tensor(
                                ug[b][:].rearrange("p (m f) -> p m f", m=M),
                                xv[:, :, 0:FEA], 0.0,
                                lv_[:, :, 0:FEA], ALU.min, ALU.subtract)
                        for b in range(NB):
                            nc.scalar.activation(ug[b][:], ug[b][:], AF.Exp)
                        for b in range(NB):
                            xv = xm[b][:].rearrange("p (m f) -> p m f", m=M)
                            lv_ = lt[b][:].rearrange("p (m f) -> p m f",
                                                     m=M)
                            sp = work.tile([128, M * FEA], bf16, tag="sps")
                            spv = sp[:].rearrange("p (m f) -> p m f", m=M)
                            nc.vector.scalar_tensor_tensor(
                                spv, xv[:, :, FEA:128], 0.0,
                                lv_[:, :, FEA:128], ALU.max, ALU.add)
                            nc.vector.tensor_tensor(ug[b][:], ug[b][:],
                                                    sp[:], ALU.mult)
                            u_ = ug[b]
                            ns = work.tile([128, FEA], f32, tag="ns")
                            nc.vector.tensor_tensor(
                                u_[:, 0:6 * FEA], u_[:, 0:6 * FEA],
                                u_[:, 6 * FEA:12 * FEA], ALU.add)
                            nc.vector.tensor_tensor(
                                u_[:, 0:3 * FEA], u_[:, 0:3 * FEA],
                                u_[:, 3 * FEA:6 * FEA], ALU.add)
                            nc.vector.tensor_tensor(
                                u_[:, 0:FEA], u_[:, 0:FEA],
                                u_[:, FEA:2 * FEA], ALU.add)
                            nc.vector.tensor_tensor(
                                ns[:], u_[:, 0:FEA],
                                u_[:, 2 * FEA:3 * FEA], ALU.add)
                            att[b] = cvp.tile([128, FEA], f32,
                                              tag=f"at{b}",
                                              name=f"at{L}{b}")
                            nc.vector.tensor_tensor(att[b][:], aprev[b][:],
                                                    ns[:], ALU.add)
                        # phase 6 (ACT batched): atom softplus
                        for b in range(NB):
                            spa[b] = cvp.tile([128, FEA], f32,
                                              tag=f"spa{b}",
                                              name=f"spa{L}{b}")
                            nc.scalar.activation(spa[b][:], att[b][:],
                                                 AF.Abs)
                        for b in range(NB):
                            nc.scalar.activation(spa[b][:], spa[b][:],
                                                 AF.Exp, scale=-1.0)
                        for b in range(NB):
                            nc.scalar.activation(spa[b][:], spa[b][:],
                                                 AF.Ln, bias=1.0)
                        for b in range(NB):
                            nc.vector.scalar_tensor_tensor(
                                anext[b][:], att[b][:], 0.0, spa[b][:],
                                ALU.max, ALU.add)
                        if L == 0:
                            # hi2 from atom1 (ones row -> exact bias fold)
                            a1x = epool.tile([FEA + 1, NL], f32, tag="a1x")
                            nc.vector.memset(a1x[FEA:FEA + 1, :], 1.0)
                            for b in range(NB):
                                sl = slice(b * 128, (b + 1) * 128)
                                tp = psCg.tile([FEA, 128], f32, tag="tpC")
                                nc.tensor.transpose(tp[:], atom1[b][:],
                                                    ident[:])
                                nc.scalar.activation(a1x[0:FEA, sl], tp[:],
                                                     AF.Copy)
                                hp = psCg.tile([128, 2 * FEA], f32,
                                               tag="hpC")
                                nc.tensor.matmul(hp[:], a1x[:, sl], wib2[:],
                                                 start=True, stop=True)
                                nc.vector.tensor_copy(hi2[b][:], hp[:])
                            if debug:
                                for b in range(NB):
                                    nc.sync.dma_start(
                                        dbg["atom1"].ap()
                                        [b * 128:(b + 1) * 128, :],
                                        atom1[b][:])

                for b in range(NB):
                    nc.sync.dma_start(d_out.ap()[b * 128:(b + 1) * 128, :],
                                      atom2[b][:])

    _body()
    nc.compile()
    return nc


def _prep_inputs(inputs):
    """Host-side layout prep. Returns (in_maps, host_ctx)."""
    import ml_dtypes
    bf = ml_dtypes.bfloat16
    f32 = np.float32
    lat = np.asarray(inputs["lat_pred"], f32)
    fr = np.ascontiguousarray(np.asarray(inputs["fracs_pred"], f32))
    sl = np.ascontiguousarray(np.asarray(inputs["species_logits"], f32))
    occ = np.asarray(inputs["occ_logits"], f32)
    emb_w = np.asarray(inputs["emb_w"], f32)
    emb_b = np.asarray(inputs["emb_b"], f32)
    w1 = np.asarray(inputs["w1"], f32); b1 = np.asarray(inputs["b1"], f32)
    w2 = np.asarray(inputs["w2"], f32); b2 = np.asarray(inputs["b2"], f32)

    G = (lat.astype(np.float64) @ lat.T.astype(np.float64))
    wroot = np.sqrt(np.diag(G)).astype(f32)

    frecs = np.zeros((N, 64), f32)
    frecs[:, 0:3] = fr

    gneg = (-np.array([G[0, 0], G[1, 1], G[2, 2],
                       2 * G[0, 1], 2 * G[0, 2], 2 * G[1, 2]])).astype(f32)

    splogT = np.zeros((128, N), f32)
    splogT[0:SPECIES, :] = sl.T

    uTh = np.zeros((36, N), f32)
    uTh[0:3, :] = np.cos(2 * np.pi * fr.T)
    uTh[32:35, :] = np.sin(2 * np.pi * fr.T)
    uTh = uTh.astype(bf)

    embwx = np.concatenate([emb_w, np.ones((SPECIES, 1), f32)], 1)
    # hi1 path: psum = af_un@wi1 + rs*(b1 + emb_b@wi1); * (1/rs) gives
    # (af_un/rs + emb_b)@wi1 + b1 = af@wi1 + b1 exactly.
    wib1 = np.ascontiguousarray(np.concatenate(
        [w1[0:FEA, :], (b1 + emb_b @ w1[0:FEA, :])[None, :]], 0))
    wib2 = np.ascontiguousarray(
        np.concatenate([w2[0:FEA, :], b2[None, :]], 0))
    wjx = np.ascontiguousarray(
        np.concatenate([w1[FEA:2 * FEA, :], w2[FEA:2 * FEA, :]], 1)).astype(bf)
    wnx = np.ascontiguousarray(
        np.concatenate([w1[2 * FEA:, :], w2[2 * FEA:, :]], 1)).astype(bf)

    shared = dict(
        splogT=splogT,
        uTh=uTh,
        frecs=frecs,
        embwx=np.ascontiguousarray(embwx),
        embbrow=np.ascontiguousarray(np.broadcast_to(emb_b, (128, FEA))),
        wib1=wib1, wib2=wib2, wjx=wjx, wnx=wnx,
        gcol=np.ascontiguousarray(np.broadcast_to(gneg, (128, 6))),
        wroot=wroot.reshape(3, 1),
        noff=(-OFFSET).reshape(KG, 1),
        blockoff=np.ascontiguousarray(np.broadcast_to(
            np.repeat(np.arange(NBLK, dtype=np.uint32) * BLK, 8),
            (128, CAND))).astype(np.uint32),
        identb=np.eye(128, dtype=f32).astype(bf),
        ident=np.eye(128, dtype=f32),
    )
    in_maps = []
    for c in range(NCORES):
        rows = slice(c * NL, (c + 1) * NL)
        selfid = (c * NL + np.arange(128, dtype=f32)[:, None]
                  + 128 * np.arange(NB, dtype=f32)[None, :]).astype(f32)
        sploclT = np.zeros((128, NL), f32)
        sploclT[0:SPECIES, :] = sl[rows].T
        m = dict(shared)
        ulocal = np.zeros((36, NL), f32)
        ulocal[0:3, :] = np.diag(G).astype(f32)[:, None] * \
            np.cos(2 * np.pi * fr[rows].T)
        ulocal[32:35, :] = np.diag(G).astype(f32)[:, None] * \
            np.sin(2 * np.pi * fr[rows].T)
        m.update(sploclT=sploclT, fl=np.ascontiguousarray(fr[rows]),
                 uloch=ulocal.astype(bf),
                 selfid=np.ascontiguousarray(selfid))
        in_maps.append(m)
    host = dict(occ=occ, fc_w=np.asarray(inputs["fc_w"], f32),
                fc_b=np.asarray(inputs["fc_b"], f32))
    return in_maps, host


def _host_finish(results, host):
    a2 = np.concatenate([np.asarray(r["atom2"]) for r in results], 0)
    occp = 1.0 / (1.0 + np.exp(-host["occ"].astype(np.float64)))
    graph = (a2.astype(np.float64) * occp[:, None]).sum(0) / (occp.sum()
                                                              + 1e-6)
    out = graph @ host["fc_w"].astype(np.float64) + host["fc_b"]
    return out.astype(np.float32)


def kernel(**inputs) -> np.ndarray:
    from concourse import bass_utils

    in_maps, host = _prep_inputs(inputs)
    key = "prog"
    if key not in _cache:
        _cache[key] = _build_program(debug=False)
    nc = _cache[key]
    res = bass_utils.run_bass_kernel_spmd(nc, in_maps,
                                          core_ids=list(range(NCORES)))
    return _host_finish(res.results, host)
